# revision 1
# baseline (speedup 1.0000x reference)
"""Trainium2 Bass kernel for a gated bilinear-attention GNN (GAT-with-gate).

Math (per batch b):
    h   = x @ W_w.T + W_b                      [N, D]
    e   = (h A) h^T ; e_sym = e + e^T = h (A + A^T) h^T   (one quadratic form)
    m   = where(adj > 0, e_sym, 0)
    att = softmax(m, axis=1) * adj             (masked entries contribute exp(0)=1
                                                to the denominator, then re-masked)
    rv  = h; 3x: az = relu(att @ rv);  c = sigmoid([h, az] @ gate_w.T + gate_b)
               rv = c * h + (1 - c) * az

Device strategy: data-parallel over the batch dim, 2 batches per core on 8
cores.  All large tensors live in a transposed layout [j(=contraction/softmax
column), i] so the softmax denominator is a per-partition (free-axis) reduction
and the hop matmul az^T = rv^T-style contraction streams at full float32r rate:

    attT[j, i] = adj[i, j] * exp(e_sym[j, i])   unnormalized, built as
                 exp((e+C)*adjT - C) so masked entries underflow to ~1e-26
    denom[j]   = exp-accum row sums + (N - indeg[j]) metadata
    azT[f, i]  = sum_j (rv[j, f]/denom[j]) * attT[j, i]   (normalization and
                 the gate coefficients folded into the stationary operand)

The sigmoid gate is evaluated as 1/(1+exp(-x)) to keep every ScalarE
activation in one LUT set (no ACT table reloads).  The two batches per core
are traced phase-interleaved so each batch's matmul bursts fill the other's
gate/combine gaps.  _fixup_waits post-processes the scheduled program to
satisfy this walrus build's one-sync-wait-per-instruction limit.

Host side only re-lays-out inputs (shard, transpose, degree metadata).
"""

import sys
from contextlib import ExitStack

import numpy as np

sys.path.insert(0, "/opt/trn_rl_repo")

import concourse.bass as bass
import concourse.tile as tile
from concourse import mybir
from concourse.bass_utils import run_bass_kernel_spmd


B, N, D = 16, 1024, 128
NCORES = 8
BPC = B // NCORES        # batches per core
NB = N // 128            # 128-row blocks per matrix dim
F32 = mybir.dt.float32
F32R = mybir.dt.float32r
OP = mybir.AluOpType
AF = mybir.ActivationFunctionType
CBIG = 60.0




def build_nc():
    nc = bass.Bass("TRN2", target_bir_lowering=False, debug=False,
                   num_devices=NCORES)

    adjT = nc.dram_tensor("adjT", [BPC, N, N], F32, kind="ExternalInput").ap()
    xT = nc.dram_tensor("xT", [BPC, D, N], F32, kind="ExternalInput").ap()
    ndegT = nc.dram_tensor("ndegT", [BPC, D, NB], F32, kind="ExternalInput").ap()
    WwT = nc.dram_tensor("WwT", [D, D], F32, kind="ExternalInput").ap()
    Wb = nc.dram_tensor("Wb", [D, 1], F32, kind="ExternalInput").ap()
    Amat = nc.dram_tensor("Amat", [D, D], F32, kind="ExternalInput").ap()
    gwc = nc.dram_tensor("gwc", [D, 2], F32, kind="ExternalInput").ap()
    gbv = nc.dram_tensor("gbv", [1, 1], F32, kind="ExternalInput").ap()
    identd = nc.dram_tensor("identd", [128, 128], F32, kind="ExternalInput").ap()
    out = nc.dram_tensor("out", [BPC, N, D], F32, kind="ExternalOutput").ap()

    with tile.TileContext(nc) as tc, ExitStack() as ctx:
        consts = ctx.enter_context(tc.tile_pool(name="consts", bufs=1))
        ps_a = ctx.enter_context(tc.tile_pool(name="ps_a", bufs=4, space="PSUM"))
        ps_az = ps_a
        ps_tr = ctx.enter_context(tc.tile_pool(name="ps_tr", bufs=2, space="PSUM"))
        ps_g = ctx.enter_context(tc.tile_pool(name="ps_g", bufs=1, space="PSUM"))
        ps_ct = ctx.enter_context(tc.tile_pool(name="ps_ct", bufs=1, space="PSUM"))
        adj_pool = ctx.enter_context(tc.tile_pool(name="adj", bufs=6))
        att_pool = ctx.enter_context(tc.tile_pool(name="att", bufs=2))
        work = ctx.enter_context(tc.tile_pool(name="work", bufs=2))
        hop = ctx.enter_context(tc.tile_pool(name="hop", bufs=4))

        ident = consts.tile([128, 128], F32, tag="ident")
        nc.sync.dma_start(ident[:, :], identd[:, :])
        wwT_sb = consts.tile([D, D], F32, tag="wwT")
        nc.sync.dma_start(wwT_sb[:, :], WwT[:, :])
        wb_sb = consts.tile([D, 1], F32, tag="wb")
        nc.sync.dma_start(wb_sb[:, :], Wb[:, :])
        a_sb = consts.tile([D, D], F32, tag="amat")
        nc.sync.dma_start(a_sb[:, :], Amat[:, :])
        gwc_sb = consts.tile([D, 2], F32, tag="gwc")
        nc.sync.dma_start(gwc_sb[:, :], gwc[:, :])
        gb_sb = consts.tile([1, 1], F32, tag="gb")
        nc.sync.dma_start(gb_sb[:, :], gbv[:, :])
        negc_sb = consts.tile([128, 1], F32, tag="negc")
        nc.vector.memset(negc_sb[:, :], -CBIG)
        ngb_sb = consts.tile([1, 1], F32, tag="ngb")
        nc.vector.tensor_scalar(ngb_sb[:, :], gb_sb[:, :], -1.0, None, OP.mult)

        identr = consts.tile([128, 128], F32R, tag="identr")
        nc.vector.tensor_copy(identr[:, :], ident[:, :])
        gwr_sb = consts.tile([D, 2], F32R, tag="gwr")
        nc.vector.tensor_copy(gwr_sb[:, :], gwc_sb[:, :])

        # PE warm-up: ~4us of tiny filler transposes during the otherwise
        # idle DMA-bound startup, so the HAM clock gate is already at 2.4GHz
        # when the first real matmuls issue.
        warm_ps = ps_ct.tile([128, NB], F32, tag="ps_ct")
        for _ in range(20):
            nc.tensor.transpose(warm_ps[:, 0:8], ident[0:8, :], ident[0:8, 0:8])

        # S = A + A^T (stays for the whole kernel)
        s_sb = consts.tile([D, D], F32R, tag="smat")
        at_ps = ps_tr.tile([128, 512], F32, tag="ps_tr")
        nc.tensor.transpose(at_ps[:, 0:128], a_sb[:, :], ident[:, :])
        nc.vector.tensor_tensor(s_sb[:, :], a_sb[:, :], at_ps[:, 0:128], OP.add)

        def phase_prologue(b, st):
            xT_sb = work.tile([D, N], F32, tag="xT")
            for ih in range(2):
                nc.sync.dma_start(xT_sb[:, ih * 512:(ih + 1) * 512],
                                  xT[b, :, ih * 512:(ih + 1) * 512])
            ndeg_sb = work.tile([D, NB], F32, tag="ndeg")
            nc.sync.dma_start(ndeg_sb[:, :], ndegT[b, :, :])

            # hT[o, n] = sum_d WwT[d, o] xT[d, n] + Wb[o]  (plain fp32
            # matmul: rhs comes straight from DMA, off the startup path)
            hT_sb = work.tile([D, N], F32R, tag="hT")
            for ih in range(2):
                ph = ps_a.tile([128, 512], F32, tag="ps_a")
                nc.tensor.matmul(ph[:, :], (wwT_sb[:, :]),
                                 (xT_sb[:, ih * 512:(ih + 1) * 512]),
                                 start=True, stop=True)
                nc.scalar.activation(hT_sb[:, ih * 512:(ih + 1) * 512], ph[:, :],
                                     AF.Identity, bias=wb_sb[:, :], scale=1.0)

            # hST[e, n] = sum_o S[o, e] hT[o, n]   (S symmetric)
            hST_sb = work.tile([D, N], F32R, tag="hST")
            for ih in range(2):
                ph = ps_a.tile([128, 512], F32, tag="ps_a")
                nc.tensor.matmul(ph[:, :], (s_sb[:, :]),
                                 (hT_sb[:, ih * 512:(ih + 1) * 512]),
                                 start=True, stop=True)
                nc.scalar.copy(hST_sb[:, ih * 512:(ih + 1) * 512], ph[:, :])

            # h in natural layout [node-in-block, nb*128 + f]
            hnat_sb = work.tile([128, N], F32, tag="hnat")
            for half in range(2):
                pt = ps_tr.tile([128, 512], F32R, tag="ps_tr")
                for q in range(4):
                    nb = half * 4 + q
                    nc.tensor.transpose(pt[:, q * 128:(q + 1) * 128],
                                        hT_sb[:, nb * 128:(nb + 1) * 128],
                                        identr[:, :])
                nc.scalar.copy(hnat_sb[:, half * 512:(half + 1) * 512],
                               pt[:, :])
            st.update(hT=hT_sb, hST=hST_sb, hnat=hnat_sb, ndeg=ndeg_sb)

        def phase_att(b, st):
            # attT = adj^T * exp(e_sym) via the masked-offset trick:
            # m = (e + C)*adjT, then exp(m - C).  Unmasked entries give
            # exp(e); masked give exp(-C) ~ 1e-26 ~ 0.  The exp's fused
            # accum_out yields sum_i over unmasked entries; reference
            # semantics add exp(0)=1 per masked entry, supplied as N-deg
            # metadata (ndegT).
            hT_sb, hST_sb = st["hT"], st["hST"]
            attT_sb = att_pool.tile([128, NB * N], F32R, tag="att")
            acc_sb = work.tile([D, NB], F32, tag="acc")
            for jb in range(NB):
                adj_sb = adj_pool.tile([128, N], F32, tag="adj")
                for ih in range(2):
                    nc.sync.dma_start(
                        adj_sb[:, ih * 512:(ih + 1) * 512],
                        adjT[b, jb * 128:(jb + 1) * 128,
                             ih * 512:(ih + 1) * 512])
                for ih in range(2):
                    pe = ps_a.tile([128, 512], F32, tag="ps_a")
                    nc.tensor.matmul(pe[:, :],
                                     (hST_sb[:, jb * 128:(jb + 1) * 128]),
                                     (hT_sb[:, ih * 512:(ih + 1) * 512]),
                                     start=True, stop=True)
                    seg = attT_sb[:, jb * N + ih * 512: jb * N + (ih + 1) * 512]
                    nc.vector.scalar_tensor_tensor(
                        seg, pe[:, :], CBIG,
                        adj_sb[:, ih * 512:(ih + 1) * 512],
                        OP.add, OP.mult)
                slab = attT_sb[:, jb * N:(jb + 1) * N]
                nc.scalar.activation(slab, slab, AF.Exp, bias=negc_sb[:, :],
                                     accum_out=acc_sb[:, jb:jb + 1])

            # denom = masked-exp row sums + (N - deg);  inv = 1/denom
            inv_sb = work.tile([D, NB], F32, tag="inv")
            nc.vector.tensor_tensor(inv_sb[:, :], acc_sb[:, :],
                                    st["ndeg"][:, :], OP.add)
            nc.vector.reciprocal(inv_sb[:, :], inv_sb[:, :])

            # rv scaled by 1/denom for the first hop's stationary operand
            rvs = hop.tile([128, N], F32R, tag="rvs")
            hnat_sb = st["hnat"]
            for nb in range(NB):
                nc.vector.tensor_scalar_mul(rvs[:, nb * 128:(nb + 1) * 128],
                                            hnat_sb[:, nb * 128:(nb + 1) * 128],
                                            inv_sb[:, nb:nb + 1])
            st.update(att=attT_sb, inv=inv_sb, rvs=rvs)

        def phase_hop(b, st, k):
            last = (k == 2)
            hT_sb, hnat_sb = st["hT"], st["hnat"]
            attT_sb, inv_sb, rvs = st["att"], st["inv"], st["rvs"]
            # azT[f, i] = sum_j rvs[j, f] attT[j, i]
            azT_sb = hop.tile([128, N], F32R, tag="azT")
            for ih in range(2):
                paz = ps_az.tile([128, 512], F32, tag="ps_a")
                for jb in range(NB):
                    nc.tensor.matmul(
                        paz[:, :], (rvs[:, jb * 128:(jb + 1) * 128]),
                        (attT_sb[:, jb * N + ih * 512: jb * N + (ih + 1) * 512]),
                        start=(jb == 0), stop=(jb == NB - 1))
                nc.scalar.activation(azT_sb[:, ih * 512:(ih + 1) * 512],
                                     paz[:, :], AF.Relu)

            # gate: coeff = sigmoid(gw1.h + gw2.az + gb) per node, computed
            # as 1/(1 + exp(-pre)) to stay in the exp LUT set (a Sigmoid
            # activation would force an ACT table swap).
            en_sb = hop.tile([1, N], F32, tag="coeff")
            for ih in range(2):
                pg = ps_g.tile([1, 512], F32, tag="ps_g")
                nc.tensor.matmul(pg[:, :], (gwr_sb[:, 0:1]),
                                 (hT_sb[:, ih * 512:(ih + 1) * 512]),
                                 start=True, stop=False)
                nc.tensor.matmul(pg[:, :], (gwr_sb[:, 1:2]),
                                 (azT_sb[:, ih * 512:(ih + 1) * 512]),
                                 start=False, stop=True)
                nc.scalar.activation(en_sb[:, ih * 512:(ih + 1) * 512],
                                     pg[:, :], AF.Exp, bias=ngb_sb[:, :],
                                     scale=-1.0)

            # transpose exp(-pre) to per-partition scalars, finish the
            # sigmoid there (tiny [128, NB] ops)
            ct_ps = ps_ct.tile([128, NB], F32, tag="ps_ct")
            for nb in range(NB):
                nc.tensor.transpose(ct_ps[:, nb:nb + 1],
                                    en_sb[0:1, nb * 128:(nb + 1) * 128],
                                    ident[0:1, 0:1])
            # coeff c = 1/(1+e); w1 = c (*1/denom unless last),
            # w2 = 1-c = e*c (*1/denom unless last)
            w1 = hop.tile([128, NB], F32, tag="w1")
            w2 = hop.tile([128, NB], F32, tag="w2")
            nc.vector.tensor_scalar(w1[:, :], ct_ps[:, :], 1.0, None, OP.add)
            nc.vector.reciprocal(w1[:, :], w1[:, :])
            nc.vector.tensor_tensor(w2[:, :], ct_ps[:, :], w1[:, :], OP.mult)
            if not last:
                nc.vector.tensor_tensor(w1[:, :], w1[:, :], inv_sb[:, :],
                                        OP.mult)
                nc.vector.tensor_tensor(w2[:, :], w2[:, :], inv_sb[:, :],
                                        OP.mult)

            # az back to natural layout, scale by w2, combine with h
            rv_new = hop.tile([128, N], F32 if last else F32R, tag="rvs")
            azs = hop.tile([128, N], F32, tag="azs")
            for half in range(2):
                pt = ps_tr.tile([128, 512], F32R, tag="ps_tr")
                for q in range(4):
                    nb = half * 4 + q
                    nc.tensor.transpose(pt[:, q * 128:(q + 1) * 128],
                                        azT_sb[:, nb * 128:(nb + 1) * 128],
                                        identr[:, :])
                for q in range(4):
                    nb = half * 4 + q
                    sl = slice(nb * 128, (nb + 1) * 128)
                    nc.vector.tensor_scalar_mul(
                        azs[:, sl], pt[:, q * 128:(q + 1) * 128],
                        w2[:, nb:nb + 1])
                    nc.vector.scalar_tensor_tensor(rv_new[:, sl],
                                                   hnat_sb[:, sl],
                                                   w1[:, nb:nb + 1],
                                                   azs[:, sl],
                                                   OP.mult, OP.add)
            if last:
                for nb in range(NB):
                    nc.sync.dma_start(out[b, nb * 128:(nb + 1) * 128, :],
                                      rv_new[:, nb * 128:(nb + 1) * 128])
            else:
                st["rvs"] = rv_new

        # Interleave the two batches phase-by-phase so each batch's PE-heavy
        # bursts fill the other batch's gate/combine gaps (keeps the PE HAM
        # clock warm and every engine fed).
        states = [{} for _ in range(BPC)]
        for b in range(BPC):
            phase_prologue(b, states[b])
        for b in range(BPC):
            phase_att(b, states[b])
        for k in range(3):
            for b in range(BPC):
                phase_hop(b, states[b], k)

        # Spare per-engine nops: relocated by _fixup_waits to carry sync
        # waits that walrus cannot fit on compute-instruction structs.
        nop_insts = []
        for eng in (nc.tensor, nc.vector, nc.scalar, nc.gpsimd, nc.sync):
            for _ in range(96):
                nop_insts.append(eng.nop(nofuse=True).ins)

    _fixup_waits(nc, nop_insts)
    return nc


_FIXUP_SKIP = {"InstNoOp"}


def _fixup_waits(nc, nop_insts):
    """walrus (enable-ldw-opt=false) rejects compute instructions with more
    than one sync wait (single wait slot in the S3 structs).  Hoist
    all-but-one wait of each such instruction onto spare same-engine nop
    instructions inserted immediately before it in program order."""
    nop_set = set(id(x) for x in nop_insts)
    free_nops = {}
    for x in nop_insts:
        free_nops.setdefault(x.engine, []).append(x)
    f = nc.m.functions[0]
    for blk in f.blocks:
        insts = blk.instructions
        for i in range(len(insts) - 1, -1, -1):
            if id(insts[i]) in nop_set:
                insts.pop(i)
        i = 0
        while i < len(insts):
            inst = insts[i]
            if inst.__class__.__name__ not in _FIXUP_SKIP:
                si = inst.sync_info
                if si is not None and si.on_wait and len(si.on_wait) > 1:
                    waits = list(si.on_wait)
                    extra, keep = waits[:-1], waits[-1:]
                    inst.sync_info = mybir.SyncInfo(
                        on_wait=keep, on_update=list(si.on_update or []))
                    pool = free_nops.get(inst.engine)
                    for k, w in enumerate(extra):
                        if not pool:
                            raise RuntimeError(
                                f"out of spare nops for {inst.engine}")
                        nop = pool.pop()
                        nop.sync_info = mybir.SyncInfo(on_wait=[w], on_update=[])
                        insts.insert(i + k, nop)
                    i += len(extra)
            i += 1


_NC_CACHE = None


def _get_nc():
    global _NC_CACHE
    if _NC_CACHE is None:
        _NC_CACHE = build_nc()
    return _NC_CACHE


def _prep_in_maps(inputs):
    x = np.ascontiguousarray(np.asarray(inputs["x"], dtype=np.float32))
    adj = np.ascontiguousarray(np.asarray(inputs["adj"], dtype=np.float32))
    W_w = np.asarray(inputs["W_w"], dtype=np.float32)
    W_b = np.asarray(inputs["W_b"], dtype=np.float32)
    A = np.asarray(inputs["A"], dtype=np.float32)
    gate_w = np.asarray(inputs["gate_w"], dtype=np.float32)
    gate_b = np.asarray(inputs["gate_b"], dtype=np.float32)

    WwT = np.ascontiguousarray(W_w.T)
    Wb2 = np.ascontiguousarray(W_b.reshape(D, 1))
    gwcols = np.ascontiguousarray(gate_w.reshape(2, D).T)
    gb2 = np.ascontiguousarray(gate_b.reshape(1, 1))
    ident128 = np.eye(128, dtype=np.float32)

    in_maps = []
    for c in range(NCORES):
        sl = slice(c * BPC, (c + 1) * BPC)
        adj_c = adj[sl]
        adjT_c = np.ascontiguousarray(adj_c.transpose(0, 2, 1))
        xT_c = np.ascontiguousarray(x[sl].transpose(0, 2, 1))
        ndeg = (N - adj_c.sum(axis=1)).astype(np.float32)          # [BPC, N]
        ndegT = np.ascontiguousarray(
            ndeg.reshape(BPC, NB, 128).transpose(0, 2, 1))         # [BPC, 128, NB]
        in_maps.append({
            "adjT": adjT_c, "xT": xT_c, "ndegT": ndegT,
            "WwT": WwT, "Wb": Wb2, "Amat": np.ascontiguousarray(A),
            "gwc": gwcols, "gbv": gb2, "identd": ident128,
        })
    return in_maps


def _run(inputs, trace=False, **kwargs):
    nc = _get_nc()
    in_maps = _prep_in_maps(inputs)
    res = run_bass_kernel_spmd(nc, in_maps, core_ids=list(range(NCORES)),
                               trace=trace, **kwargs)
    out = np.concatenate([res.results[c]["out"] for c in range(NCORES)], axis=0)
    return out.astype(np.float32), res


def kernel(**inputs) -> np.ndarray:
    out, _ = _run(inputs, trace=False)
    return out



# revision 12
# speedup vs baseline: 1.0651x; 1.0651x over previous
"""Trainium2 Bass kernel for a gated bilinear-attention GNN (GAT-with-gate).

Math (per batch b):
    h   = x @ W_w.T + W_b                      [N, D]
    e   = (h A) h^T ; e_sym = e + e^T = h (A + A^T) h^T   (one quadratic form)
    m   = where(adj > 0, e_sym, 0)
    att = softmax(m, axis=1) * adj             (masked entries contribute exp(0)=1
                                                to the denominator, then re-masked)
    rv  = h; 3x: az = relu(att @ rv);  c = sigmoid([h, az] @ gate_w.T + gate_b)
               rv = c * h + (1 - c) * az

Device strategy: data-parallel over the batch dim, 2 batches per core on 8
cores.  v3 design vs the f32r baseline:

  * All matmul operands bf16 (f32 PSUM accumulate): 1 row/cycle PE rate,
    half the SBUF/HBM traffic, lower PE power.  Full-bf16 pipeline rel err
    ~2.7e-3 (numpy-simulated) vs the 2e-2 gate.
  * adj ships as bf16 (exact 0/1 mask): halves the dominant DMA stream.
  * hST = (A+A^T)-transformed h comes from its own matmul with the
    host-folded stationary WST = W_w.T @ (A+A^T) (and bias SWb), so it
    reads xT like the h matmul instead of chaining on hT.
  * Layout changes run on the DMA xbar, not the PE: hT -> h-natural and
    azT -> az-natural are ONE dma transpose instruction each ([128,1024]
    bf16, block-wise via a 3D out AP).
  * Masking happens after the exp: ACT exps e straight out of PSUM into
    the bf16 attT slab; one DVE tensor_tensor_reduce then multiplies by
    adjT in place AND accumulates the softmax denominator, seeded with the
    (N - indegree) metadata as the reduction's initial value.  Masked
    entries are exactly 0.  max|e| ~ 12 here so exp never overflows.
  * The gate runs on the PE as a [*,16]-stationary matmul producing 16
    identical rows of gate pre-activations (same cycles as one row), ACT
    exps them, and one xbar transpose turns the [16, N] row block into
    [128, 8x16] per-partition coefficient columns - no 48 tiny PE
    transposes, no per-element gate work on DVE/Pool.
  * Sigmoid stays in exp form (1/(1+exp(-x))) so ACT never reloads its
    LUT table set; w1 = inv/(1+en) and w2 = inv - w1 are tiny [128,128]
    DVE ops.
  * Denominator reciprocals are computed per half-batch so the first
    hop's az accumulation (over j-blocks, in order) chases the attention
    pipeline instead of waiting for the full softmax.

Host side only re-lays-out inputs (shard, transpose, bf16 cast, degree
metadata, folded weights).  _fixup_waits post-processes the scheduled
program to satisfy this walrus build's one-sync-wait-per-instruction limit.
"""

import sys
from contextlib import ExitStack

import numpy as np

sys.path.insert(0, "/opt/trn_rl_repo")

import concourse.bass as bass
import concourse.tile as tile
from concourse import mybir
from concourse.bass_utils import run_bass_kernel_spmd

import ml_dtypes


B, N, D = 16, 1024, 128
NCORES = 8
BPC = B // NCORES        # batches per core
NB = N // 128            # 128-row blocks per matrix dim
F32 = mybir.dt.float32
BF16 = mybir.dt.bfloat16
OP = mybir.AluOpType
AF = mybir.ActivationFunctionType


def build_nc():
    nc = bass.Bass("TRN2", target_bir_lowering=False, debug=False,
                   num_devices=NCORES)

    adjT = nc.dram_tensor("adjT", [BPC, N, N], BF16, kind="ExternalInput").ap()
    xT = nc.dram_tensor("xT", [BPC, D, N], BF16, kind="ExternalInput").ap()
    ndegT = nc.dram_tensor("ndegT", [BPC, D, NB], F32, kind="ExternalInput").ap()
    WwT = nc.dram_tensor("WwT", [D, D], BF16, kind="ExternalInput").ap()
    WST = nc.dram_tensor("WST", [D, D], BF16, kind="ExternalInput").ap()
    Wb = nc.dram_tensor("Wb", [D, 1], F32, kind="ExternalInput").ap()
    SWb = nc.dram_tensor("SWb", [D, 1], F32, kind="ExternalInput").ap()
    gw1c = nc.dram_tensor("gw1c", [D, 16], BF16, kind="ExternalInput").ap()
    gw2c = nc.dram_tensor("gw2c", [D, 16], BF16, kind="ExternalInput").ap()
    ngb = nc.dram_tensor("ngb", [16, 1], F32, kind="ExternalInput").ap()
    out = nc.dram_tensor("out", [BPC, N, D], F32, kind="ExternalOutput").ap()

    with tile.TileContext(nc) as tc, ExitStack() as ctx:
        consts = ctx.enter_context(tc.tile_pool(name="consts", bufs=1))
        pse = ctx.enter_context(tc.tile_pool(name="pse", bufs=2, space="PSUM"))
        psa = ctx.enter_context(tc.tile_pool(name="psa", bufs=2, space="PSUM"))
        psg = ctx.enter_context(tc.tile_pool(name="psg", bufs=2, space="PSUM"))
        adj_pool = ctx.enter_context(tc.tile_pool(name="adj", bufs=3))
        att_pool = ctx.enter_context(tc.tile_pool(name="att", bufs=2))
        work = ctx.enter_context(tc.tile_pool(name="work", bufs=2))
        hop = ctx.enter_context(tc.tile_pool(name="hop", bufs=3))
        rv_pool = ctx.enter_context(tc.tile_pool(name="rv", bufs=4))

        wwT_sb = consts.tile([D, D], BF16, tag="wwT")
        nc.sync.dma_start(wwT_sb[:, :], WwT[:, :])
        wst_sb = consts.tile([D, D], BF16, tag="wst")
        nc.sync.dma_start(wst_sb[:, :], WST[:, :])
        wb_sb = consts.tile([D, 1], F32, tag="wb")
        nc.sync.dma_start(wb_sb[:, :], Wb[:, :])
        swb_sb = consts.tile([D, 1], F32, tag="swb")
        nc.sync.dma_start(swb_sb[:, :], SWb[:, :])
        gw1_sb = consts.tile([D, 16], BF16, tag="gw1c")
        nc.sync.dma_start(gw1_sb[:, :], gw1c[:, :])
        gw2_sb = consts.tile([D, 16], BF16, tag="gw2c")
        nc.sync.dma_start(gw2_sb[:, :], gw2c[:, :])
        ngb_sb = consts.tile([16, 1], F32, tag="ngb")
        nc.sync.dma_start(ngb_sb[:, :], ngb[:, :])
        ones16 = consts.tile([128, 16], F32, tag="ones16")
        nc.vector.memset(ones16[:, :], 1.0)

        # PE warm-up: small matmuls on the first-arrived weight tile keep the
        # PE p-state ramped during the otherwise DMA-bound startup.
        for _ in range(12):
            wps = psg.tile([16, 512], F32, tag="psg")
            nc.tensor.matmul(wps[:, 0:128], wwT_sb[:, 0:16], wwT_sb[:, :],
                             start=True, stop=True)

        def phase_prologue(b, st):
            xT_sb = work.tile([D, N], BF16, tag="xT")
            nc.sync.dma_start(xT_sb[:, :], xT[b, :, :])
            ndeg_sb = work.tile([D, NB], F32, tag="ndeg")
            nc.sync.dma_start(ndeg_sb[:, :], ndegT[b, :, :])

            # hT[o, n] = sum_d WwT[d, o] xT[d, n] + Wb[o]
            # hST[e, n] = sum_d WST[d, e] xT[d, n] + SWb[e]
            hT_sb = work.tile([D, N], BF16, tag="hT")
            hST_sb = work.tile([D, N], BF16, tag="hST")
            for dst, wmat, bias in ((hT_sb, wwT_sb, wb_sb),
                                    (hST_sb, wst_sb, swb_sb)):
                for ih in range(2):
                    ph = psa.tile([128, 512], F32, tag="psa")
                    nc.tensor.matmul(ph[:, :], wmat[:, :],
                                     xT_sb[:, ih * 512:(ih + 1) * 512],
                                     start=True, stop=True)
                    nc.scalar.activation(dst[:, ih * 512:(ih + 1) * 512],
                                         ph[:, :], AF.Identity,
                                         bias=bias[:, :], scale=1.0)

            # h natural layout via one xbar transpose:
            # hnat[p, nb*128+f] = hT[f, nb*128+p]
            hnat_sb = work.tile([128, N], BF16, tag="hnat")
            nc.sync.dma_start_transpose(
                hnat_sb[:, :].rearrange("p (nb f) -> p nb f", nb=NB),
                hT_sb[:, :])
            st.update(hT=hT_sb, hST=hST_sb, hnat=hnat_sb, ndeg=ndeg_sb)

        def phase_att(b, st):
            # attT[k, j] = adj[j, k] * exp(e_sym[k, j]).  ACT exps PSUM into
            # the bf16 slab; DVE ttr masks in place and accumulates the
            # denominator, seeded with (N - indegree).
            hT_sb, hST_sb, ndeg_sb = st["hT"], st["hST"], st["ndeg"]
            hnat_sb = st["hnat"]
            attT_sb = att_pool.tile([128, NB * N], BF16, tag="att")
            acc_sb = work.tile([D, NB], F32, tag="acc")
            inv_sb = work.tile([D, NB], F32, tag="inv")
            rvs = rv_pool.tile([128, N], BF16, tag="rvs")
            for jb in range(NB):
                adj_sb = adj_pool.tile([128, N], BF16, tag="adj")
                nc.sync.dma_start(adj_sb[:, :],
                                  adjT[b, jb * 128:(jb + 1) * 128, :])
                pe = pse.tile([128, N], F32, tag="pse")
                for ih in range(2):
                    nc.tensor.matmul(pe[:, ih * 512:(ih + 1) * 512],
                                     hST_sb[:, jb * 128:(jb + 1) * 128],
                                     hT_sb[:, ih * 512:(ih + 1) * 512],
                                     start=True, stop=True)
                slab = attT_sb[:, jb * N:(jb + 1) * N]
                nc.scalar.activation(slab, pe[:, :], AF.Exp)
                nc.vector.scalar_tensor_tensor(
                    slab, slab, 1.0, adj_sb[:, :], OP.mult, OP.mult,
                    accum_out=acc_sb[:, jb:jb + 1])
                if jb in (3, 7):
                    # per-half reciprocals + first-hop stationary scaling so
                    # hop 0's jb-ordered accumulation can chase this pipeline
                    half = slice(jb - 3, jb + 1)
                    nc.vector.tensor_tensor(inv_sb[:, half], acc_sb[:, half],
                                            ndeg_sb[:, half], OP.add)
                    nc.vector.reciprocal(inv_sb[:, half], inv_sb[:, half])
                    for nb in range(jb - 3, jb + 1):
                        nc.vector.tensor_scalar_mul(
                            rvs[:, nb * 128:(nb + 1) * 128],
                            hnat_sb[:, nb * 128:(nb + 1) * 128],
                            inv_sb[:, nb:nb + 1])
            # inv replicated x16 to match the gate coefficient layout
            invr_sb = work.tile([128, 128], F32, tag="invr")
            for nb in range(NB):
                nc.vector.tensor_scalar_mul(invr_sb[:, nb * 16:(nb + 1) * 16],
                                            ones16[:, :],
                                            inv_sb[:, nb:nb + 1])
            st.update(att=attT_sb, invr=invr_sb, rvs=rvs)

        def phase_hop(b, st, k):
            last = (k == 2)
            hT_sb, hnat_sb = st["hT"], st["hnat"]
            attT_sb, invr_sb, rvs = st["att"], st["invr"], st["rvs"]
            # azT[f, i] = sum_j rvs[j, f] attT[j, i]; gate pre-activations as
            # 16 identical rows right behind it on the PE.
            azT_sb = hop.tile([128, N], BF16, tag="azT")
            en_sb = hop.tile([16, N], BF16, tag="en16")
            for ih in range(2):
                sl = slice(ih * 512, (ih + 1) * 512)
                paz = psa.tile([128, 512], F32, tag="psa")
                for jb in range(NB):
                    nc.tensor.matmul(
                        paz[:, :], rvs[:, jb * 128:(jb + 1) * 128],
                        attT_sb[:, jb * N + ih * 512: jb * N + (ih + 1) * 512],
                        start=(jb == 0), stop=(jb == NB - 1))
                nc.scalar.activation(azT_sb[:, sl], paz[:, :], AF.Relu)
                pg = psg.tile([16, 512], F32, tag="psg")
                nc.tensor.matmul(pg[:, :], gw1_sb[:, :], hT_sb[:, sl],
                                 start=True, stop=False)
                nc.tensor.matmul(pg[:, :], gw2_sb[:, :], azT_sb[:, sl],
                                 start=False, stop=True)
                nc.scalar.activation(en_sb[:, sl], pg[:, :], AF.Exp,
                                     bias=ngb_sb[:, :], scale=-1.0)

            # [16, N] row block -> [128, 8x16] coefficient columns (one xbar)
            enc_sb = hop.tile([128, 128], BF16, tag="enc")
            nc.sync.dma_start_transpose(
                enc_sb[:, :].rearrange("p (nb r) -> p nb r", nb=NB),
                en_sb[:, :])
            # az natural via one xbar transpose
            aznat_sb = hop.tile([128, N], BF16, tag="aznat")
            nc.sync.dma_start_transpose(
                aznat_sb[:, :].rearrange("p (nb f) -> p nb f", nb=NB),
                azT_sb[:, :])

            # w1 = c = 1/(1+en), w2 = 1-c, both times 1/denom unless last
            w1_sb = hop.tile([128, 128], F32, tag="w1")
            w2_sb = hop.tile([128, 128], F32, tag="w2")
            nc.vector.tensor_scalar(w1_sb[:, :], enc_sb[:, :], 1.0, None,
                                    OP.add)
            nc.vector.reciprocal(w1_sb[:, :], w1_sb[:, :])
            if last:
                nc.vector.tensor_scalar(w2_sb[:, :], w1_sb[:, :], -1.0, 1.0,
                                        OP.mult, OP.add)
            else:
                nc.vector.tensor_tensor(w1_sb[:, :], w1_sb[:, :],
                                        invr_sb[:, :], OP.mult)
                nc.vector.tensor_tensor(w2_sb[:, :], invr_sb[:, :],
                                        w1_sb[:, :], OP.subtract)

            # combine: rv_new = hnat*w1 + aznat*w2  (per node-block scalars)
            rv_new = rv_pool.tile([128, N], F32 if last else BF16,
                                  tag="rvout" if last else "rvs")
            azs = hop.tile([128, N], BF16, tag="azs")
            for nb in range(NB):
                sl = slice(nb * 128, (nb + 1) * 128)
                nc.vector.tensor_scalar_mul(azs[:, sl], aznat_sb[:, sl],
                                            w2_sb[:, nb * 16:nb * 16 + 1])
                nc.vector.scalar_tensor_tensor(rv_new[:, sl], hnat_sb[:, sl],
                                               w1_sb[:, nb * 16:nb * 16 + 1],
                                               azs[:, sl], OP.mult, OP.add)
            if last:
                nc.sync.dma_start(
                    out[b].rearrange("(nb p) f -> p nb f", p=128),
                    rv_new[:, :].rearrange("p (nb f) -> p nb f", nb=NB))
            else:
                st["rvs"] = rv_new

        # Interleave the two batches phase-by-phase so each batch's PE-heavy
        # bursts fill the other batch's gate/combine gaps.
        states = [{} for _ in range(BPC)]
        for b in range(BPC):
            phase_prologue(b, states[b])
        for b in range(BPC):
            phase_att(b, states[b])
        for k in range(3):
            for b in range(BPC):
                phase_hop(b, states[b], k)

        # Spare per-engine nops: relocated by _fixup_waits to carry sync
        # waits that walrus cannot fit on compute-instruction structs.
        nop_insts = []
        for eng in (nc.tensor, nc.vector, nc.scalar, nc.gpsimd, nc.sync):
            for _ in range(128):
                nop_insts.append(eng.nop(nofuse=True).ins)

    _fixup_waits(nc, nop_insts)
    return nc


_FIXUP_SKIP = {"InstNoOp"}


def _fixup_waits(nc, nop_insts):
    """walrus (enable-ldw-opt=false) rejects compute instructions with more
    than one sync wait (single wait slot in the S3 structs).  Hoist
    all-but-one wait of each such instruction onto spare same-engine nop
    instructions inserted immediately before it in program order."""
    nop_set = set(id(x) for x in nop_insts)
    free_nops = {}
    for x in nop_insts:
        free_nops.setdefault(x.engine, []).append(x)
    f = nc.m.functions[0]
    for blk in f.blocks:
        insts = blk.instructions
        for i in range(len(insts) - 1, -1, -1):
            if id(insts[i]) in nop_set:
                insts.pop(i)
        i = 0
        while i < len(insts):
            inst = insts[i]
            if inst.__class__.__name__ not in _FIXUP_SKIP:
                si = inst.sync_info
                if si is not None and si.on_wait and len(si.on_wait) > 1:
                    waits = list(si.on_wait)
                    extra, keep = waits[:-1], waits[-1:]
                    inst.sync_info = mybir.SyncInfo(
                        on_wait=keep, on_update=list(si.on_update or []))
                    pool = free_nops.get(inst.engine)
                    for kk, w in enumerate(extra):
                        if not pool:
                            raise RuntimeError(
                                f"out of spare nops for {inst.engine}")
                        nop = pool.pop()
                        nop.sync_info = mybir.SyncInfo(on_wait=[w], on_update=[])
                        insts.insert(i + kk, nop)
                    i += len(extra)
            i += 1


_NC_CACHE = None


def _get_nc():
    global _NC_CACHE
    if _NC_CACHE is None:
        _NC_CACHE = build_nc()
    return _NC_CACHE


def _bf16(a):
    return np.ascontiguousarray(
        np.asarray(a, dtype=np.float32).astype(ml_dtypes.bfloat16))


def _prep_in_maps(inputs):
    x = np.ascontiguousarray(np.asarray(inputs["x"], dtype=np.float32))
    adj = np.ascontiguousarray(np.asarray(inputs["adj"], dtype=np.float32))
    W_w = np.asarray(inputs["W_w"], dtype=np.float32)
    W_b = np.asarray(inputs["W_b"], dtype=np.float32)
    A = np.asarray(inputs["A"], dtype=np.float32)
    gate_w = np.asarray(inputs["gate_w"], dtype=np.float32)
    gate_b = np.asarray(inputs["gate_b"], dtype=np.float32)

    S = A + A.T
    WwT = _bf16(W_w.T)
    WST = _bf16(W_w.T @ S)
    Wb2 = np.ascontiguousarray(W_b.reshape(D, 1))
    SWb = np.ascontiguousarray((S @ W_b).reshape(D, 1), dtype=np.float32)
    gw1 = _bf16(np.broadcast_to(gate_w[0, :D].reshape(D, 1), (D, 16)))
    gw2 = _bf16(np.broadcast_to(gate_w[0, D:].reshape(D, 1), (D, 16)))
    ngb = np.ascontiguousarray(
        np.broadcast_to(-gate_b.reshape(1, 1), (16, 1)), dtype=np.float32)

    in_maps = []
    for c in range(NCORES):
        sl = slice(c * BPC, (c + 1) * BPC)
        adj_c = adj[sl]
        adjT_c = _bf16(adj_c.transpose(0, 2, 1))
        xT_c = _bf16(x[sl].transpose(0, 2, 1))
        ndeg = (N - adj_c.sum(axis=1)).astype(np.float32)          # [BPC, N]
        ndegT = np.ascontiguousarray(
            ndeg.reshape(BPC, NB, 128).transpose(0, 2, 1))         # [BPC, 128, NB]
        in_maps.append({
            "adjT": adjT_c, "xT": xT_c, "ndegT": ndegT,
            "WwT": WwT, "WST": WST, "Wb": Wb2, "SWb": SWb,
            "gw1c": gw1, "gw2c": gw2, "ngb": ngb,
        })
    return in_maps


def _run(inputs, trace=False, **kwargs):
    nc = _get_nc()
    in_maps = _prep_in_maps(inputs)
    res = run_bass_kernel_spmd(nc, in_maps, core_ids=list(range(NCORES)),
                               trace=trace, **kwargs)
    out = np.concatenate([res.results[c]["out"] for c in range(NCORES)], axis=0)
    return out.astype(np.float32), res


def kernel(**inputs) -> np.ndarray:
    out, _ = _run(inputs, trace=False)
    return out


# revision 16
# speedup vs baseline: 1.1763x; 1.1044x over previous
"""Trainium2 Bass kernel for a gated bilinear-attention GNN (GAT-with-gate).

Math (per batch b):
    h   = x @ W_w.T + W_b                      [N, D]
    e   = h (A + A^T) h^T  (symmetrized bilinear score, one quadratic form)
    att = softmax(where(adj>0, e, 0), axis=1) * adj
    rv  = h; 3x: az = relu(att @ rv);  c = sigmoid([h, az] @ gate_w.T + gate_b)
               rv = c * h + (1 - c) * az

Device strategy: data-parallel over the batch dim, 2 batches per core on 8
cores.  v4 design notes:

  * All matmul operands bf16 (f32 PSUM accumulate): 1 row/cycle PE rate.
    Full-bf16 pipeline rel err ~2.8e-3 vs the 2e-2 gate (measured).
  * adj ships as bf16 (exact 0/1 mask).
  * hST comes from its own matmul with host-folded WST = W_w.T (A+A^T).
  * Layout changes ride the DMA xbar ([128,1024] bf16 block transposes via
    3D out APs): hT->h-natural, azT->az-natural, and the gate coefficient
    row->column turn.  The tensor engine runs ONLY matmuls.
  * Masking happens after the exp: ACT exps e from PSUM into the bf16 attT
    slab; one DVE scalar_tensor_tensor multiplies by adjT in place AND
    accumulates the softmax denominator.  attT is then normalized by
    1/denominator per partition (its partition IS the softmax index), so
    hops need no per-hop 1/denom folding and hop 0's stationary is h_nat
    itself.  max|e| ~ 12 here, exp never overflows before the mask.
  * The gate matmul uses 128-replicated-column stationaries, so sigmoid
    pre-activations come out as 128 identical PSUM rows; ACT applies
    Sigmoid directly (one LUT swap for the whole kernel: all att exps
    precede all hop sigmoids in ACT program order; Relu lives in both
    tables), and one xbar turns [128, N] replicated rows into the
    broadcast coefficient tile cbc[p, nb*128+j] = c[node nb*128+p].
  * The combine is three full-width bf16 2x DVE tensor_tensors per
    hop-batch: hma = h - az; t = hma * cbc; rv = t + az.  No per-block
    scalar ops anywhere in the hop loop.
  * Emission is interleaved batch-by-batch at op granularity (engine
    queues execute in order, so coarse phase interleave would stall).
  * Data DMAs ride the gpsimd software-DGE queue; the sync queue carries
    only the xbar transposes (which are hwdge-only and have ~1.2us fixed
    ucode cost each).

Host side only re-lays-out inputs (shard, transpose, bf16 cast, degree
metadata, folded weights).  _fixup_waits post-processes the scheduled
program to satisfy this walrus build's one-sync-wait-per-instruction limit.
"""

import sys
from contextlib import ExitStack

import numpy as np

sys.path.insert(0, "/opt/trn_rl_repo")

import concourse.bass as bass
import concourse.tile as tile
from concourse import mybir
from concourse.bass_utils import run_bass_kernel_spmd

import ml_dtypes


B, N, D = 16, 1024, 128
NCORES = 8
BPC = B // NCORES        # batches per core
NB = N // 128            # 128-row blocks per matrix dim
F32 = mybir.dt.float32
BF16 = mybir.dt.bfloat16
OP = mybir.AluOpType
AF = mybir.ActivationFunctionType


def build_nc():
    nc = bass.Bass("TRN2", target_bir_lowering=False, debug=False,
                   num_devices=NCORES)

    adjT = nc.dram_tensor("adjT", [BPC, N, N], BF16, kind="ExternalInput").ap()
    xT = nc.dram_tensor("xT", [BPC, D, N], BF16, kind="ExternalInput").ap()
    ndegT = nc.dram_tensor("ndegT", [D, BPC * NB], F32, kind="ExternalInput").ap()
    # packed consts: [WwT | WST | gw1c128 | gw2c128]
    cb = nc.dram_tensor("cb", [D, 4 * D], BF16, kind="ExternalInput").ap()
    # packed f32 consts: [Wb | SWb | gb]
    cf = nc.dram_tensor("cf", [D, 3], F32, kind="ExternalInput").ap()
    out = nc.dram_tensor("out", [BPC, N, D], F32, kind="ExternalOutput").ap()

    with tile.TileContext(nc) as tc, ExitStack() as ctx:
        consts = ctx.enter_context(tc.tile_pool(name="consts", bufs=1))
        pse = ctx.enter_context(tc.tile_pool(name="pse", bufs=2, space="PSUM"))
        psa = ctx.enter_context(tc.tile_pool(name="psa", bufs=2, space="PSUM"))
        psg = ctx.enter_context(tc.tile_pool(name="psg", bufs=2, space="PSUM"))
        adj_pool = ctx.enter_context(tc.tile_pool(name="adj", bufs=4))
        att_pool = ctx.enter_context(tc.tile_pool(name="att", bufs=2))
        work = ctx.enter_context(tc.tile_pool(name="work", bufs=2))
        hop = ctx.enter_context(tc.tile_pool(name="hop", bufs=3))
        rv_pool = ctx.enter_context(tc.tile_pool(name="rv", bufs=4))

        cb_sb = consts.tile([D, 4 * D], BF16, tag="cb")
        nc.gpsimd.dma_start(cb_sb[:, :], cb[:, :])
        cf_sb = consts.tile([D, 3], F32, tag="cf")
        nc.gpsimd.dma_start(cf_sb[:, :], cf[:, :])
        wwT_sb = cb_sb[:, 0:D]
        wst_sb = cb_sb[:, D:2 * D]
        gw1_sb = cb_sb[:, 2 * D:3 * D]
        gw2_sb = cb_sb[:, 3 * D:4 * D]
        wb_sb = cf_sb[:, 0:1]
        swb_sb = cf_sb[:, 1:2]
        gb_sb = cf_sb[:, 2:3]

        # PE warm-up on the packed const tile: keeps the PE p-state ramped
        # during the DMA-bound startup.
        for _ in range(12):
            wps = psg.tile([128, 512], F32, tag="psg")
            nc.tensor.matmul(wps[:, 0:128], wwT_sb[:, :], wwT_sb[:, :],
                             start=True, stop=True)

        states = [{} for _ in range(BPC)]

        def phase_prologue():
            for b in range(BPC):
                st = states[b]
                xT_sb = work.tile([D, N], BF16, tag="xT")
                nc.gpsimd.dma_start(xT_sb[:, :], xT[b, :, :])
                st["xT"] = xT_sb
            ndeg_sb = consts.tile([D, BPC * NB], F32, tag="ndeg")
            nc.gpsimd.dma_start(ndeg_sb[:, :], ndegT[:, :])
            for b in range(BPC):
                st = states[b]
                st["ndeg"] = ndeg_sb[:, b * NB:(b + 1) * NB]
                # hT[o,n] = WwT^T x + Wb ; hST[e,n] = WST^T x + SWb
                hT_sb = work.tile([D, N], BF16, tag="hT")
                hST_sb = work.tile([D, N], BF16, tag="hST")
                for dst, wmat, bias in ((hT_sb, wwT_sb, wb_sb),
                                        (hST_sb, wst_sb, swb_sb)):
                    for ih in range(2):
                        ph = psa.tile([128, 512], F32, tag="psa")
                        nc.tensor.matmul(ph[:, :], wmat,
                                         st["xT"][:, ih * 512:(ih + 1) * 512],
                                         start=True, stop=True)
                        nc.scalar.activation(dst[:, ih * 512:(ih + 1) * 512],
                                             ph[:, :], AF.Identity,
                                             bias=bias, scale=1.0)
                hnat_sb = work.tile([128, N], BF16, tag="hnat")
                nc.sync.dma_start_transpose(
                    hnat_sb[:, :].rearrange("p (nb f) -> p nb f", nb=NB),
                    hT_sb[:, :])
                st.update(hT=hT_sb, hST=hST_sb, hnat=hnat_sb)

        def phase_att():
            # attT[k, j] = adj[j, k] exp(e[k, j]) / denom[k]:
            # exp on ACT straight from PSUM, in-place mask + denominator
            # accumulate on DVE, then per-partition normalize (partition IS
            # the softmax index k).  Batches interleave op-by-op.
            for b in range(BPC):
                st = states[b]
                att_sb = att_pool.tile([128, NB * N], BF16, tag="att")
                acc_sb = work.tile([D, NB], F32, tag="acc")
                inv_sb = work.tile([D, NB], F32, tag="inv")
                st.update(att=att_sb, acc=acc_sb, inv=inv_sb)
            for jb in range(NB):
                for b in range(BPC):
                    st = states[b]
                    adj_sb = adj_pool.tile([128, N], BF16, tag="adj")
                    nc.gpsimd.dma_start(adj_sb[:, :],
                                        adjT[b, jb * 128:(jb + 1) * 128, :])
                    pe = pse.tile([128, N], F32, tag="pse")
                    for ih in range(2):
                        nc.tensor.matmul(pe[:, ih * 512:(ih + 1) * 512],
                                         st["hST"][:, jb * 128:(jb + 1) * 128],
                                         st["hT"][:, ih * 512:(ih + 1) * 512],
                                         start=True, stop=True)
                    slab = st["att"][:, jb * N:(jb + 1) * N]
                    nc.scalar.activation(slab, pe[:, :], AF.Exp)
                    nc.vector.scalar_tensor_tensor(
                        slab, slab, 1.0, adj_sb[:, :], OP.mult, OP.mult,
                        accum_out=st["acc"][:, jb:jb + 1])
                if jb in (3, 7):
                    lo = jb - 3
                    for b in range(BPC):
                        st = states[b]
                        half = slice(lo, jb + 1)
                        nc.vector.tensor_tensor(st["inv"][:, half],
                                                st["acc"][:, half],
                                                st["ndeg"][:, half], OP.add)
                        nc.vector.reciprocal(st["inv"][:, half],
                                             st["inv"][:, half])
                        for nb in range(lo, jb + 1):
                            nc.vector.tensor_scalar_mul(
                                st["att"][:, nb * N:(nb + 1) * N],
                                st["att"][:, nb * N:(nb + 1) * N],
                                st["inv"][:, nb:nb + 1])

        def phase_hop(k):
            last = (k == 2)
            for b in range(BPC):
                st = states[b]
                rvs = st.get("rvs") or st["hnat"]
                azT_sb = hop.tile([128, N], BF16, tag="azT")
                st["azT"] = azT_sb
                for ih in range(2):
                    sl = slice(ih * 512, (ih + 1) * 512)
                    paz = psa.tile([128, 512], F32, tag="psa")
                    for jb in range(NB):
                        nc.tensor.matmul(
                            paz[:, :], rvs[:, jb * 128:(jb + 1) * 128],
                            st["att"][:, jb * N + ih * 512:
                                      jb * N + (ih + 1) * 512],
                            start=(jb == 0), stop=(jb == NB - 1))
                    if ih == 0:
                        nc.scalar.activation(azT_sb[:, sl], paz[:, :], AF.Relu)
                    else:
                        nc.vector.tensor_scalar(azT_sb[:, sl], paz[:, :],
                                                0.0, None, OP.max)
                    pg = psg.tile([128, 512], F32, tag="psg")
                    nc.tensor.matmul(pg[:, :], gw1_sb, st["hT"][:, sl],
                                     start=True, stop=False)
                    nc.tensor.matmul(pg[:, :], gw2_sb, azT_sb[:, sl],
                                     start=False, stop=True)
                    if ih == 0:
                        c128_sb = hop.tile([128, N], BF16, tag="c128")
                        st["c128"] = c128_sb
                    nc.scalar.activation(st["c128"][:, sl], pg[:, :],
                                         AF.Sigmoid, bias=gb_sb, scale=1.0)
            for b in range(BPC):
                st = states[b]
                aznat_sb = hop.tile([128, N], BF16, tag="aznat")
                st["aznat"] = aznat_sb
                nc.sync.dma_start_transpose(
                    aznat_sb[:, :].rearrange("p (nb f) -> p nb f", nb=NB),
                    st["azT"][:, :])
            for b in range(BPC):
                st = states[b]
                cbc_sb = hop.tile([128, N], BF16, tag="cbc")
                nc.sync.dma_start_transpose(
                    cbc_sb[:, :].rearrange("p (nb j) -> p nb j", nb=NB),
                    st["c128"][:, :])
                st["cbc"] = cbc_sb
            for b in range(BPC):
                st = states[b]
                hnat_sb, aznat_sb, cbc_sb = st["hnat"], st["aznat"], st["cbc"]
                hma_sb = hop.tile([128, N], BF16, tag="hma")
                nc.vector.tensor_tensor(hma_sb[:, :], hnat_sb[:, :],
                                        aznat_sb[:, :], OP.subtract)
                tt_sb = hop.tile([128, N], BF16, tag="tt")
                nc.vector.tensor_tensor(tt_sb[:, :], hma_sb[:, :],
                                        cbc_sb[:, :], OP.mult)
                rv_new = rv_pool.tile([128, N], F32 if last else BF16,
                                      tag="rvout" if last else "rvs")
                nc.vector.tensor_tensor(rv_new[:, :], tt_sb[:, :],
                                        aznat_sb[:, :], OP.add)
                if last:
                    nc.gpsimd.dma_start(
                        out[b].rearrange("(nb p) f -> p nb f", p=128),
                        rv_new[:, :].rearrange("p (nb f) -> p nb f", nb=NB))
                else:
                    st["rvs"] = rv_new

        phase_prologue()
        phase_att()
        for k in range(3):
            phase_hop(k)

        # Spare per-engine nops: relocated by _fixup_waits to carry sync
        # waits that walrus cannot fit on compute-instruction structs.
        nop_insts = []
        for eng in (nc.tensor, nc.vector, nc.scalar, nc.gpsimd, nc.sync):
            for _ in range(128):
                nop_insts.append(eng.nop(nofuse=True).ins)

    _fixup_waits(nc, nop_insts)
    return nc


_FIXUP_SKIP = {"InstNoOp"}


def _fixup_waits(nc, nop_insts):
    """walrus (enable-ldw-opt=false) rejects compute instructions with more
    than one sync wait (single wait slot in the S3 structs).  Hoist
    all-but-one wait of each such instruction onto spare same-engine nop
    instructions inserted immediately before it in program order."""
    nop_set = set(id(x) for x in nop_insts)
    free_nops = {}
    for x in nop_insts:
        free_nops.setdefault(x.engine, []).append(x)
    f = nc.m.functions[0]
    for blk in f.blocks:
        insts = blk.instructions
        for i in range(len(insts) - 1, -1, -1):
            if id(insts[i]) in nop_set:
                insts.pop(i)
        i = 0
        while i < len(insts):
            inst = insts[i]
            if inst.__class__.__name__ not in _FIXUP_SKIP:
                si = inst.sync_info
                if si is not None and si.on_wait and len(si.on_wait) > 1:
                    waits = list(si.on_wait)
                    extra, keep = waits[:-1], waits[-1:]
                    inst.sync_info = mybir.SyncInfo(
                        on_wait=keep, on_update=list(si.on_update or []))
                    pool = free_nops.get(inst.engine)
                    for kk, w in enumerate(extra):
                        if not pool:
                            raise RuntimeError(
                                f"out of spare nops for {inst.engine}")
                        nop = pool.pop()
                        nop.sync_info = mybir.SyncInfo(on_wait=[w], on_update=[])
                        insts.insert(i + kk, nop)
                    i += len(extra)
            i += 1


_NC_CACHE = None


def _get_nc():
    global _NC_CACHE
    if _NC_CACHE is None:
        _NC_CACHE = build_nc()
    return _NC_CACHE


def _bf16(a):
    return np.ascontiguousarray(
        np.asarray(a, dtype=np.float32).astype(ml_dtypes.bfloat16))


def _prep_in_maps(inputs):
    x = np.ascontiguousarray(np.asarray(inputs["x"], dtype=np.float32))
    adj = np.ascontiguousarray(np.asarray(inputs["adj"], dtype=np.float32))
    W_w = np.asarray(inputs["W_w"], dtype=np.float32)
    W_b = np.asarray(inputs["W_b"], dtype=np.float32)
    A = np.asarray(inputs["A"], dtype=np.float32)
    gate_w = np.asarray(inputs["gate_w"], dtype=np.float32)
    gate_b = np.asarray(inputs["gate_b"], dtype=np.float32)

    S = A + A.T
    cb = np.concatenate([
        W_w.T,
        W_w.T @ S,
        np.broadcast_to(gate_w[0, :D].reshape(D, 1), (D, D)),
        np.broadcast_to(gate_w[0, D:].reshape(D, 1), (D, D)),
    ], axis=1)
    cb = _bf16(cb)
    cf = np.ascontiguousarray(
        np.stack([W_b, S @ W_b, np.full(D, gate_b[0])], axis=1),
        dtype=np.float32)

    in_maps = []
    for c in range(NCORES):
        sl = slice(c * BPC, (c + 1) * BPC)
        adj_c = adj[sl]
        adjT_c = _bf16(adj_c.transpose(0, 2, 1))
        xT_c = _bf16(x[sl].transpose(0, 2, 1))
        ndeg = (N - adj_c.sum(axis=1)).astype(np.float32)          # [BPC, N]
        ndegT = np.ascontiguousarray(
            ndeg.reshape(BPC * NB, 128).T)                         # [128, BPC*NB]
        in_maps.append({
            "adjT": adjT_c, "xT": xT_c, "ndegT": ndegT,
            "cb": cb, "cf": cf,
        })
    return in_maps


def _run(inputs, trace=False, **kwargs):
    nc = _get_nc()
    in_maps = _prep_in_maps(inputs)
    res = run_bass_kernel_spmd(nc, in_maps, core_ids=list(range(NCORES)),
                               trace=trace, **kwargs)
    out = np.concatenate([res.results[c]["out"] for c in range(NCORES)], axis=0)
    return out.astype(np.float32), res


def kernel(**inputs) -> np.ndarray:
    out, _ = _run(inputs, trace=False)
    return out


# revision 17
# speedup vs baseline: 1.1784x; 1.0018x over previous
"""Trainium2 Bass kernel for a gated bilinear-attention GNN (GAT-with-gate).

Math (per batch b):
    h   = x @ W_w.T + W_b                      [N, D]
    e   = h (A + A^T) h^T  (symmetrized bilinear score, one quadratic form)
    att = softmax(where(adj>0, e, 0), axis=1) * adj
    rv  = h; 3x: az = relu(att @ rv);  c = sigmoid([h, az] @ gate_w.T + gate_b)
               rv = c * h + (1 - c) * az

Device strategy: data-parallel over the batch dim, 2 batches per core on 8
cores.  v5 design notes:

  * All matmul operands bf16 (f32 PSUM accumulate): 1 row/cycle PE rate.
  * adj ships as bf16 (exact 0/1 mask); x, weights bf16; output bf16
    (host casts to f32).  Measured rel err ~4e-3 vs the 2e-2 gate.
  * hST comes from its own matmul with host-folded WST = W_w.T (A+A^T).
  * Masking happens after the exp: ACT exps e from PSUM into the bf16 attT
    slab; one DVE scalar_tensor_tensor per slab multiplies by adjT in
    place AND accumulates the softmax denominator.  attT is then
    normalized per partition (its partition IS the softmax output index),
    so hops never touch 1/denom and hop 0's stationary is h-natural.
  * The gate matmul uses 128-replicated-column stationaries: sigmoid
    pre-activations appear as 128 identical PSUM rows and ACT applies
    Sigmoid once per batch-hop on [128, N].  In TRANSPOSED space that
    replicated c128 tile IS the broadcast coefficient, so the whole hop
    combine is three full-width bf16 DVE tensor_tensors per batch:
        hmaT = hT - azT;  ttT = hmaT * c128;  rvT = ttT + azT
    and ONE xbar dma transpose turns rvT into the next hop's natural-
    layout stationary.  (ACT does one LUT swap for the whole kernel:
    all att exps precede all sigmoids in ACT program order; Relu lives
    in both tables.)
  * Emission interleaves the two batches op-by-op (engine queues are
    strictly in-order).
  * Data DMAs ride the gpsimd software-DGE queue; the sync queue carries
    only the 8 xbar transposes (~1.3us fixed ucode cost each).

Host side only re-lays-out inputs (shard, transpose, bf16 cast, degree
metadata, folded weights).  _fixup_waits post-processes the scheduled
program to satisfy this walrus build's one-sync-wait-per-instruction limit.
"""

import sys
from contextlib import ExitStack

import numpy as np

sys.path.insert(0, "/opt/trn_rl_repo")

import concourse.bass as bass
import concourse.tile as tile
from concourse import mybir
from concourse.bass_utils import run_bass_kernel_spmd

import ml_dtypes


B, N, D = 16, 1024, 128
NCORES = 8
BPC = B // NCORES        # batches per core
NB = N // 128            # 128-row blocks per matrix dim
F32 = mybir.dt.float32
BF16 = mybir.dt.bfloat16
OP = mybir.AluOpType
AF = mybir.ActivationFunctionType


def build_nc():
    nc = bass.Bass("TRN2", target_bir_lowering=False, debug=False,
                   num_devices=NCORES)

    adjT = nc.dram_tensor("adjT", [BPC, N, N], BF16, kind="ExternalInput").ap()
    xT = nc.dram_tensor("xT", [BPC, D, N], BF16, kind="ExternalInput").ap()
    ndegT = nc.dram_tensor("ndegT", [D, BPC * NB], F32, kind="ExternalInput").ap()
    # packed consts: [WwT | WST | gw1c128 | gw2c128]
    cb = nc.dram_tensor("cb", [D, 4 * D], BF16, kind="ExternalInput").ap()
    # packed f32 consts: [Wb | SWb | gb]
    cf = nc.dram_tensor("cf", [D, 3], F32, kind="ExternalInput").ap()
    out = nc.dram_tensor("out", [BPC, N, D], BF16, kind="ExternalOutput").ap()

    with tile.TileContext(nc) as tc, ExitStack() as ctx:
        consts = ctx.enter_context(tc.tile_pool(name="consts", bufs=1))
        pse = ctx.enter_context(tc.tile_pool(name="pse", bufs=2, space="PSUM"))
        psa = ctx.enter_context(tc.tile_pool(name="psa", bufs=2, space="PSUM"))
        psg = ctx.enter_context(tc.tile_pool(name="psg", bufs=1, space="PSUM"))
        adj_pool = ctx.enter_context(tc.tile_pool(name="adj", bufs=4))
        att_pool = ctx.enter_context(tc.tile_pool(name="att", bufs=2))
        work = ctx.enter_context(tc.tile_pool(name="work", bufs=2))
        hop = ctx.enter_context(tc.tile_pool(name="hop", bufs=3))
        rv_pool = ctx.enter_context(tc.tile_pool(name="rv", bufs=4))

        cb_sb = consts.tile([D, 4 * D], BF16, tag="cb")
        nc.gpsimd.dma_start(cb_sb[:, :], cb[:, :])
        cf_sb = consts.tile([D, 3], F32, tag="cf")
        nc.gpsimd.dma_start(cf_sb[:, :], cf[:, :])
        wwT_sb = cb_sb[:, 0:D]
        wst_sb = cb_sb[:, D:2 * D]
        gw1_sb = cb_sb[:, 2 * D:3 * D]
        gw2_sb = cb_sb[:, 3 * D:4 * D]
        wb_sb = cf_sb[:, 0:1]
        swb_sb = cf_sb[:, 1:2]
        gb_sb = cf_sb[:, 2:3]

        # PE warm-up on the packed const tile: keeps the PE p-state ramped
        # during the DMA-bound startup.
        for _ in range(12):
            wps = psg.tile([128, N], F32, tag="psg")
            nc.tensor.matmul(wps[:, 0:128], wwT_sb[:, :], wwT_sb[:, :],
                             start=True, stop=True)

        states = [{} for _ in range(BPC)]

        def phase_prologue():
            for b in range(BPC):
                st = states[b]
                xT_sb = work.tile([D, N], BF16, tag="xT")
                nc.gpsimd.dma_start(xT_sb[:, :], xT[b, :, :])
                st["xT"] = xT_sb
            ndeg_sb = consts.tile([D, BPC * NB], F32, tag="ndeg")
            nc.gpsimd.dma_start(ndeg_sb[:, :], ndegT[:, :])
            for b in range(BPC):
                st = states[b]
                st["ndeg"] = ndeg_sb[:, b * NB:(b + 1) * NB]
                # hT[o,n] = WwT^T x + Wb ; hST[e,n] = WST^T x + SWb
                hT_sb = work.tile([D, N], BF16, tag="hT")
                hST_sb = work.tile([D, N], BF16, tag="hST")
                for dst, wmat, bias in ((hT_sb, wwT_sb, wb_sb),
                                        (hST_sb, wst_sb, swb_sb)):
                    ph = pse.tile([128, N], F32, tag="pse")
                    for ih in range(2):
                        nc.tensor.matmul(ph[:, ih * 512:(ih + 1) * 512], wmat,
                                         st["xT"][:, ih * 512:(ih + 1) * 512],
                                         start=True, stop=True)
                    nc.scalar.activation(dst[:, :], ph[:, :], AF.Identity,
                                         bias=bias, scale=1.0)
                hnat_sb = work.tile([128, N], BF16, tag="hnat")
                nc.sync.dma_start_transpose(
                    hnat_sb[:, :].rearrange("p (nb f) -> p nb f", nb=NB),
                    hT_sb[:, :])
                st.update(hT=hT_sb, hST=hST_sb, hnat=hnat_sb)

        def phase_att():
            # attT[k, j] = adj[j, k] exp(e[k, j]) / denom[k]:
            # exp on ACT straight from PSUM, in-place mask + denominator
            # accumulate on DVE, then per-partition normalize (partition IS
            # the softmax index k).  Batches interleave op-by-op.
            for b in range(BPC):
                st = states[b]
                att_sb = att_pool.tile([128, NB * N], BF16, tag="att")
                acc_sb = work.tile([D, NB], F32, tag="acc")
                inv_sb = work.tile([D, NB], F32, tag="inv")
                st.update(att=att_sb, acc=acc_sb, inv=inv_sb)
            for jb in range(NB):
                for b in range(BPC):
                    st = states[b]
                    adj_sb = adj_pool.tile([128, N], BF16, tag="adj")
                    nc.gpsimd.dma_start(adj_sb[:, :],
                                        adjT[b, jb * 128:(jb + 1) * 128, :])
                    pe = pse.tile([128, N], F32, tag="pse")
                    for ih in range(2):
                        nc.tensor.matmul(pe[:, ih * 512:(ih + 1) * 512],
                                         st["hST"][:, jb * 128:(jb + 1) * 128],
                                         st["hT"][:, ih * 512:(ih + 1) * 512],
                                         start=True, stop=True)
                    slab = st["att"][:, jb * N:(jb + 1) * N]
                    nc.scalar.activation(slab, pe[:, :], AF.Exp)
                    nc.vector.scalar_tensor_tensor(
                        slab, slab, 1.0, adj_sb[:, :], OP.mult, OP.mult,
                        accum_out=st["acc"][:, jb:jb + 1])
                if jb in (3, 7):
                    lo = jb - 3
                    for b in range(BPC):
                        st = states[b]
                        half = slice(lo, jb + 1)
                        nc.vector.tensor_tensor(st["inv"][:, half],
                                                st["acc"][:, half],
                                                st["ndeg"][:, half], OP.add)
                        nc.vector.reciprocal(st["inv"][:, half],
                                             st["inv"][:, half])
                        for nb in range(lo, jb + 1):
                            nc.vector.tensor_scalar_mul(
                                st["att"][:, nb * N:(nb + 1) * N],
                                st["att"][:, nb * N:(nb + 1) * N],
                                st["inv"][:, nb:nb + 1])

        def phase_hop(k):
            last = (k == 2)
            # PE per batch: az accumulation (2x 8 matmuls) then the gate
            # (replicated-column stationaries into one [128, N] psum).
            for b in range(BPC):
                st = states[b]
                rvs = st.get("rvs") or st["hnat"]
                azT_sb = hop.tile([128, N], BF16, tag="azT")
                st["azT"] = azT_sb
                pg = psg.tile([128, N], F32, tag="psg")
                for ih in range(2):
                    sl = slice(ih * 512, (ih + 1) * 512)
                    paz = psa.tile([128, 512], F32, tag="psa")
                    for jb in range(NB):
                        nc.tensor.matmul(
                            paz[:, :], rvs[:, jb * 128:(jb + 1) * 128],
                            st["att"][:, jb * N + ih * 512:
                                      jb * N + (ih + 1) * 512],
                            start=(jb == 0), stop=(jb == NB - 1))
                    nc.scalar.activation(azT_sb[:, sl], paz[:, :], AF.Relu)
                    nc.tensor.matmul(pg[:, sl], gw1_sb, st["hT"][:, sl],
                                     start=True, stop=False)
                    nc.tensor.matmul(pg[:, sl], gw2_sb, azT_sb[:, sl],
                                     start=False, stop=True)
                c128_sb = hop.tile([128, N], BF16, tag="c128")
                st["c128"] = c128_sb
                nc.scalar.activation(c128_sb[:, :], pg[:, :], AF.Sigmoid,
                                     bias=gb_sb, scale=1.0)
            # transposed-space combine: three full-width TTs per batch, then
            # one xbar turns rvT into next hop's natural-layout stationary.
            for b in range(BPC):
                st = states[b]
                hma_sb = hop.tile([128, N], BF16, tag="hma")
                nc.vector.tensor_tensor(hma_sb[:, :], st["hT"][:, :],
                                        st["azT"][:, :], OP.subtract)
                tt_sb = hop.tile([128, N], BF16, tag="tt")
                nc.vector.tensor_tensor(tt_sb[:, :], hma_sb[:, :],
                                        st["c128"][:, :], OP.mult)
                rvT_sb = hop.tile([128, N], BF16, tag="rvT")
                nc.vector.tensor_tensor(rvT_sb[:, :], tt_sb[:, :],
                                        st["azT"][:, :], OP.add)
                rv_nat = rv_pool.tile([128, N], BF16, tag="rvs")
                nc.sync.dma_start_transpose(
                    rv_nat[:, :].rearrange("p (nb f) -> p nb f", nb=NB),
                    rvT_sb[:, :])
                if last:
                    nc.gpsimd.dma_start(
                        out[b].rearrange("(nb p) f -> p nb f", p=128),
                        rv_nat[:, :].rearrange("p (nb f) -> p nb f", nb=NB))
                else:
                    st["rvs"] = rv_nat

        phase_prologue()
        phase_att()
        for k in range(3):
            phase_hop(k)

        # Spare per-engine nops: relocated by _fixup_waits to carry sync
        # waits that walrus cannot fit on compute-instruction structs.
        nop_insts = []
        for eng in (nc.tensor, nc.vector, nc.scalar, nc.gpsimd, nc.sync):
            for _ in range(128):
                nop_insts.append(eng.nop(nofuse=True).ins)

    _fixup_waits(nc, nop_insts)
    return nc


_FIXUP_SKIP = {"InstNoOp"}


def _fixup_waits(nc, nop_insts):
    """walrus (enable-ldw-opt=false) rejects compute instructions with more
    than one sync wait (single wait slot in the S3 structs).  Hoist
    all-but-one wait of each such instruction onto spare same-engine nop
    instructions inserted immediately before it in program order."""
    nop_set = set(id(x) for x in nop_insts)
    free_nops = {}
    for x in nop_insts:
        free_nops.setdefault(x.engine, []).append(x)
    f = nc.m.functions[0]
    for blk in f.blocks:
        insts = blk.instructions
        for i in range(len(insts) - 1, -1, -1):
            if id(insts[i]) in nop_set:
                insts.pop(i)
        i = 0
        while i < len(insts):
            inst = insts[i]
            if inst.__class__.__name__ not in _FIXUP_SKIP:
                si = inst.sync_info
                if si is not None and si.on_wait and len(si.on_wait) > 1:
                    waits = list(si.on_wait)
                    extra, keep = waits[:-1], waits[-1:]
                    inst.sync_info = mybir.SyncInfo(
                        on_wait=keep, on_update=list(si.on_update or []))
                    pool = free_nops.get(inst.engine)
                    for kk, w in enumerate(extra):
                        if not pool:
                            raise RuntimeError(
                                f"out of spare nops for {inst.engine}")
                        nop = pool.pop()
                        nop.sync_info = mybir.SyncInfo(on_wait=[w], on_update=[])
                        insts.insert(i + kk, nop)
                    i += len(extra)
            i += 1


_NC_CACHE = None


def _get_nc():
    global _NC_CACHE
    if _NC_CACHE is None:
        _NC_CACHE = build_nc()
    return _NC_CACHE


def _bf16(a):
    return np.ascontiguousarray(
        np.asarray(a, dtype=np.float32).astype(ml_dtypes.bfloat16))


def _prep_in_maps(inputs):
    x = np.ascontiguousarray(np.asarray(inputs["x"], dtype=np.float32))
    adj = np.ascontiguousarray(np.asarray(inputs["adj"], dtype=np.float32))
    W_w = np.asarray(inputs["W_w"], dtype=np.float32)
    W_b = np.asarray(inputs["W_b"], dtype=np.float32)
    A = np.asarray(inputs["A"], dtype=np.float32)
    gate_w = np.asarray(inputs["gate_w"], dtype=np.float32)
    gate_b = np.asarray(inputs["gate_b"], dtype=np.float32)

    S = A + A.T
    cb = np.concatenate([
        W_w.T,
        W_w.T @ S,
        np.broadcast_to(gate_w[0, :D].reshape(D, 1), (D, D)),
        np.broadcast_to(gate_w[0, D:].reshape(D, 1), (D, D)),
    ], axis=1)
    cb = _bf16(cb)
    cf = np.ascontiguousarray(
        np.stack([W_b, S @ W_b, np.full(D, gate_b[0])], axis=1),
        dtype=np.float32)

    in_maps = []
    for c in range(NCORES):
        sl = slice(c * BPC, (c + 1) * BPC)
        adj_c = adj[sl]
        adjT_c = _bf16(adj_c.transpose(0, 2, 1))
        xT_c = _bf16(x[sl].transpose(0, 2, 1))
        ndeg = (N - adj_c.sum(axis=1)).astype(np.float32)          # [BPC, N]
        ndegT = np.ascontiguousarray(
            ndeg.reshape(BPC * NB, 128).T)                         # [128, BPC*NB]
        in_maps.append({
            "adjT": adjT_c, "xT": xT_c, "ndegT": ndegT,
            "cb": cb, "cf": cf,
        })
    return in_maps


def _run(inputs, trace=False, **kwargs):
    nc = _get_nc()
    in_maps = _prep_in_maps(inputs)
    res = run_bass_kernel_spmd(nc, in_maps, core_ids=list(range(NCORES)),
                               trace=trace, **kwargs)
    out = np.concatenate(
        [np.asarray(res.results[c]["out"]).astype(np.float32)
         for c in range(NCORES)], axis=0)
    return out, res


def kernel(**inputs) -> np.ndarray:
    out, _ = _run(inputs, trace=False)
    return out


# revision 21
# speedup vs baseline: 1.1972x; 1.0159x over previous
"""Trainium2 Bass kernel for a gated bilinear-attention GNN (GAT-with-gate).

Math (per batch b):
    h   = x @ W_w.T + W_b                      [N, D]
    e   = h (A + A^T) h^T  (symmetrized bilinear score, one quadratic form)
    att = softmax(where(adj>0, e, 0), axis=1) * adj
    rv  = h; 3x: az = relu(att @ rv);  c = sigmoid([h, az] @ gate_w.T + gate_b)
               rv = c * h + (1 - c) * az

Device strategy: data-parallel over the batch dim, 2 batches per core on 8
cores.  v5 design notes:

  * All matmul operands bf16 (f32 PSUM accumulate): 1 row/cycle PE rate.
  * adj ships as bf16 (exact 0/1 mask); x, weights bf16; output bf16
    (host casts to f32).  Measured rel err ~4e-3 vs the 2e-2 gate.
  * hST comes from its own matmul with host-folded WST = W_w.T (A+A^T).
  * Masking happens after the exp: ACT exps e from PSUM into the bf16 attT
    slab; one DVE scalar_tensor_tensor per slab multiplies by adjT in
    place AND accumulates the softmax denominator.  attT is then
    normalized per partition (its partition IS the softmax output index),
    so hops never touch 1/denom and hop 0's stationary is h-natural.
  * The gate matmul uses 128-replicated-column stationaries: sigmoid
    pre-activations appear as 128 identical PSUM rows and ACT applies
    Sigmoid once per batch-hop on [128, N].  In TRANSPOSED space that
    replicated c128 tile IS the broadcast coefficient, so the whole hop
    combine is three full-width bf16 DVE tensor_tensors per batch:
        hmaT = hT - azT;  ttT = hmaT * c128;  rvT = ttT + azT
    and ONE xbar dma transpose turns rvT into the next hop's natural-
    layout stationary.  (ACT does one LUT swap for the whole kernel:
    all att exps precede all sigmoids in ACT program order; Relu lives
    in both tables.)
  * Emission interleaves the two batches op-by-op (engine queues are
    strictly in-order).
  * Data DMAs ride the gpsimd software-DGE queue; the sync queue carries
    only the 8 xbar transposes (~1.3us fixed ucode cost each).

Host side only re-lays-out inputs (shard, transpose, bf16 cast, degree
metadata, folded weights).  _fixup_waits post-processes the scheduled
program to satisfy this walrus build's one-sync-wait-per-instruction limit.
"""

import sys
from contextlib import ExitStack

import numpy as np

sys.path.insert(0, "/opt/trn_rl_repo")

import concourse.bass as bass
import concourse.tile as tile
from concourse import mybir
from concourse.bass_utils import run_bass_kernel_spmd
import concourse.bass_utils as _bu

import ml_dtypes



B, N, D = 16, 1024, 128
NCORES = 8
BPC = B // NCORES        # batches per core
NB = N // 128            # 128-row blocks per matrix dim
F32 = mybir.dt.float32
BF16 = mybir.dt.bfloat16
OP = mybir.AluOpType
AF = mybir.ActivationFunctionType


def build_nc():
    nc = bass.Bass("TRN2", target_bir_lowering=False, debug=False,
                   num_devices=NCORES)

    adjT = nc.dram_tensor("adjT", [BPC, N, N], BF16, kind="ExternalInput").ap()
    xT = nc.dram_tensor("xT", [BPC, D, N], BF16, kind="ExternalInput").ap()
    ndegT = nc.dram_tensor("ndegT", [D, BPC * NB], F32, kind="ExternalInput").ap()
    # packed consts: [WwT | WST | gw1c128 | gw2c128]
    cb = nc.dram_tensor("cb", [D, 4 * D], BF16, kind="ExternalInput").ap()
    # packed f32 consts: [Wb | SWb | gb]
    cf = nc.dram_tensor("cf", [D, 3], F32, kind="ExternalInput").ap()
    out = nc.dram_tensor("out", [BPC, D, N], BF16, kind="ExternalOutput").ap()

    with tile.TileContext(nc) as tc, ExitStack() as ctx:
        consts = ctx.enter_context(tc.tile_pool(name="consts", bufs=1))
        pse = ctx.enter_context(tc.tile_pool(name="pse", bufs=2, space="PSUM"))
        psa = ctx.enter_context(tc.tile_pool(name="psa", bufs=2, space="PSUM"))
        psg = ctx.enter_context(tc.tile_pool(name="psg", bufs=1, space="PSUM"))
        adj_pool = ctx.enter_context(tc.tile_pool(name="adj", bufs=4))
        att_pool = ctx.enter_context(tc.tile_pool(name="att", bufs=2))
        work = ctx.enter_context(tc.tile_pool(name="work", bufs=2))
        hop = ctx.enter_context(tc.tile_pool(name="hop", bufs=3))
        rv_pool = ctx.enter_context(tc.tile_pool(name="rv", bufs=4))

        cb_sb = consts.tile([D, 4 * D], BF16, tag="cb")
        nc.gpsimd.dma_start(cb_sb[:, :], cb[:, :])
        cf_sb = consts.tile([D, 3], F32, tag="cf")
        nc.gpsimd.dma_start(cf_sb[:, :], cf[:, :])
        wwT_sb = cb_sb[:, 0:D]
        wst_sb = cb_sb[:, D:2 * D]
        gw1_sb = cb_sb[:, 2 * D:3 * D]
        gw2_sb = cb_sb[:, 3 * D:4 * D]
        wb_sb = cf_sb[:, 0:1]
        swb_sb = cf_sb[:, 1:2]
        gb_sb = cf_sb[:, 2:3]

        # PE warm-up on the packed const tile: keeps the PE p-state ramped
        # during the DMA-bound startup.
        for _ in range(12):
            wps = psg.tile([128, N], F32, tag="psg")
            nc.tensor.matmul(wps[:, 0:128], wwT_sb[:, :], wwT_sb[:, :],
                             start=True, stop=True)

        states = [{} for _ in range(BPC)]

        def phase_prologue():
            for b in range(BPC):
                st = states[b]
                xT_sb = work.tile([D, N], BF16, tag="xT")
                nc.gpsimd.dma_start(xT_sb[:, :], xT[b, :, :])
                st["xT"] = xT_sb
            ndeg_sb = consts.tile([D, BPC * NB], F32, tag="ndeg")
            nc.gpsimd.dma_start(ndeg_sb[:, :], ndegT[:, :])
            for b in range(BPC):
                st = states[b]
                st["ndeg"] = ndeg_sb[:, b * NB:(b + 1) * NB]
                # hT[o,n] = WwT^T x + Wb ; hST[e,n] = WST^T x + SWb
                hT_sb = work.tile([D, N], BF16, tag="hT")
                hST_sb = work.tile([D, N], BF16, tag="hST")
                for dst, wmat, bias in ((hT_sb, wwT_sb, wb_sb),
                                        (hST_sb, wst_sb, swb_sb)):
                    ph = pse.tile([128, N], F32, tag="pse")
                    for ih in range(2):
                        nc.tensor.matmul(ph[:, ih * 512:(ih + 1) * 512], wmat,
                                         st["xT"][:, ih * 512:(ih + 1) * 512],
                                         start=True, stop=True)
                    nc.scalar.activation(dst[:, :], ph[:, :], AF.Identity,
                                         bias=bias, scale=1.0)
                hnat_sb = work.tile([128, N], BF16, tag="hnat")
                nc.sync.dma_start_transpose(
                    hnat_sb[:, :].rearrange("p (nb f) -> p nb f", nb=NB),
                    hT_sb[:, :])
                st.update(hT=hT_sb, hST=hST_sb, hnat=hnat_sb)

        def att_gen(b):
            # attT[k, j] = adj[j, k] exp(e[k, j]) / denom[k]:
            # exp on ACT straight from PSUM, in-place mask + denominator
            # accumulate on DVE, then per-partition normalize (partition IS
            # the softmax output index).
            st = states[b]
            att_sb = att_pool.tile([128, NB * N], BF16, tag="att")
            acc_sb = work.tile([D, NB], F32, tag="acc")
            inv_sb = work.tile([D, NB], F32, tag="inv")
            st.update(att=att_sb, acc=acc_sb, inv=inv_sb)
            for jb in range(NB):
                adj_sb = adj_pool.tile([128, N], BF16, tag="adj")
                nc.gpsimd.dma_start(adj_sb[:, :],
                                    adjT[b, jb * 128:(jb + 1) * 128, :])
                pe = pse.tile([128, N], F32, tag="pse")
                for ih in range(2):
                    nc.tensor.matmul(pe[:, ih * 512:(ih + 1) * 512],
                                     st["hST"][:, jb * 128:(jb + 1) * 128],
                                     st["hT"][:, ih * 512:(ih + 1) * 512],
                                     start=True, stop=True)
                slab = att_sb[:, jb * N:(jb + 1) * N]
                nc.scalar.activation(slab, pe[:, :], AF.Exp)
                nc.vector.scalar_tensor_tensor(
                    slab, slab, 1.0, adj_sb[:, :], OP.mult, OP.mult,
                    accum_out=acc_sb[:, jb:jb + 1])
                if jb in (3, 7):
                    lo = jb - 3
                    half = slice(lo, jb + 1)
                    nc.vector.tensor_tensor(inv_sb[:, half], acc_sb[:, half],
                                            st["ndeg"][:, half], OP.add)
                    nc.vector.reciprocal(inv_sb[:, half], inv_sb[:, half])
                    for nb in range(lo, jb + 1):
                        nc.vector.tensor_scalar_mul(
                            att_sb[:, nb * N:(nb + 1) * N],
                            att_sb[:, nb * N:(nb + 1) * N],
                            inv_sb[:, nb:nb + 1])
                yield

        def hop_gen(b, k):
            last = (k == 2)
            st = states[b]
            rvs = st.get("rvs") or st["hnat"]
            azT_sb = hop.tile([128, N], BF16, tag="azT")
            for ih in range(2):
                sl = slice(ih * 512, (ih + 1) * 512)
                paz = psa.tile([128, 512], F32, tag="psa")
                for jb in range(NB):
                    nc.tensor.matmul(
                        paz[:, :], rvs[:, jb * 128:(jb + 1) * 128],
                        st["att"][:, jb * N + ih * 512:
                                  jb * N + (ih + 1) * 512],
                        start=(jb == 0), stop=(jb == NB - 1))
                nc.scalar.activation(azT_sb[:, sl], paz[:, :], AF.Relu)
                yield
            pg = psg.tile([128, N], F32, tag="psg")
            for ih in range(2):
                sl = slice(ih * 512, (ih + 1) * 512)
                nc.tensor.matmul(pg[:, sl], gw1_sb, st["hT"][:, sl],
                                 start=True, stop=False)
                nc.tensor.matmul(pg[:, sl], gw2_sb, azT_sb[:, sl],
                                 start=False, stop=True)
            c128_sb = hop.tile([128, N], BF16, tag="c128")
            nc.scalar.activation(c128_sb[:, :], pg[:, :], AF.Sigmoid,
                                 bias=gb_sb, scale=1.0)
            yield
            # transposed-space combine: c128's replicated rows ARE the
            # broadcast coefficient here.
            hma_sb = hop.tile([128, N], BF16, tag="hma")
            nc.vector.tensor_tensor(hma_sb[:, :], st["hT"][:, :],
                                    azT_sb[:, :], OP.subtract)
            tt_sb = hop.tile([128, N], BF16, tag="tt")
            nc.vector.tensor_tensor(tt_sb[:, :], hma_sb[:, :],
                                    c128_sb[:, :], OP.mult)
            rvT_sb = hop.tile([128, N], BF16, tag="rvT")
            nc.vector.tensor_tensor(rvT_sb[:, :], tt_sb[:, :],
                                    azT_sb[:, :], OP.add)
            if last:
                # output stays transposed ([D, N] per batch); host transposes
                nc.gpsimd.dma_start(out[b], rvT_sb[:, :])
            else:
                rv_nat = rv_pool.tile([128, N], BF16, tag="rvs")
                nc.sync.dma_start_transpose(
                    rv_nat[:, :].rearrange("p (nb f) -> p nb f", nb=NB),
                    rvT_sb[:, :])
                st["rvs"] = rv_nat
            yield

        def zip_run(*gens):
            gens = list(gens)
            while gens:
                for g in list(gens):
                    try:
                        next(g)
                    except StopIteration:
                        gens.remove(g)

        # software pipeline: att(b1) overlaps hop0(b0); hop(b1,k) overlaps
        # hop(b0,k+1).  Engine queues are in-order, so the overlap comes
        # from round-robin emission of the two phases' chunks.
        phase_prologue()
        zip_run(att_gen(0))
        zip_run(att_gen(1), hop_gen(0, 0))
        zip_run(hop_gen(1, 0), hop_gen(0, 1))
        zip_run(hop_gen(1, 1), hop_gen(0, 2))
        zip_run(hop_gen(1, 2))

        # Spare per-engine nops: relocated by _fixup_waits to carry sync
        # waits that walrus cannot fit on compute-instruction structs.
        nop_insts = []
        for eng in (nc.tensor, nc.vector, nc.scalar, nc.gpsimd, nc.sync):
            for _ in range(128):
                nop_insts.append(eng.nop(nofuse=True).ins)

    _fixup_waits(nc, nop_insts)
    return nc


def _fuse_ldweights(nc):
    """Remove the tile framework's pre-split InstLdweights records, merging
    their sync waits/updates into the following (self-loading) Matmult, so
    walrus --enable-ldw-opt=true can overlap stationary loads."""
    f = nc.m.functions[0]
    for blk in f.blocks:
        insts = blk.instructions
        kept = []
        pending = None
        for inst in insts:
            name = inst.__class__.__name__
            if name == "InstLdweights":
                si = inst.sync_info
                assert pending is None
                pending = (list(si.on_wait or []) if si else [],
                           list(si.on_update or []) if si else [])
                continue
            if pending is not None:
                assert name == "InstMatmult", f"ldw followed by {name}"
                w, u = pending
                si = inst.sync_info
                cw = list(si.on_wait or []) if si else []
                cu = list(si.on_update or []) if si else []
                inst.sync_info = mybir.SyncInfo(on_wait=w + cw,
                                                on_update=cu + u)
                pending = None
            kept.append(inst)
        assert pending is None
        if len(kept) != len(insts):
            insts[:] = kept


_FIXUP_SKIP = {"InstNoOp"}


def _fixup_waits(nc, nop_insts):
    """walrus (enable-ldw-opt=false) rejects compute instructions with more
    than one sync wait (single wait slot in the S3 structs).  Hoist
    all-but-one wait of each such instruction onto spare same-engine nop
    instructions inserted immediately before it in program order."""
    nop_set = set(id(x) for x in nop_insts)
    free_nops = {}
    for x in nop_insts:
        free_nops.setdefault(x.engine, []).append(x)
    f = nc.m.functions[0]
    for blk in f.blocks:
        insts = blk.instructions
        for i in range(len(insts) - 1, -1, -1):
            if id(insts[i]) in nop_set:
                insts.pop(i)
        i = 0
        while i < len(insts):
            inst = insts[i]
            if inst.__class__.__name__ not in _FIXUP_SKIP:
                si = inst.sync_info
                if si is not None and si.on_wait and len(si.on_wait) > 1:
                    waits = list(si.on_wait)
                    extra, keep = waits[:-1], waits[-1:]
                    inst.sync_info = mybir.SyncInfo(
                        on_wait=keep, on_update=list(si.on_update or []))
                    pool = free_nops.get(inst.engine)
                    for kk, w in enumerate(extra):
                        if not pool:
                            raise RuntimeError(
                                f"out of spare nops for {inst.engine}")
                        nop = pool.pop()
                        nop.sync_info = mybir.SyncInfo(on_wait=[w], on_update=[])
                        insts.insert(i + kk, nop)
                    i += len(extra)
            i += 1


_NC_CACHE = None


def _get_nc():
    global _NC_CACHE
    if _NC_CACHE is None:
        _NC_CACHE = build_nc()
    return _NC_CACHE


def _bf16(a):
    return np.ascontiguousarray(
        np.asarray(a, dtype=np.float32).astype(ml_dtypes.bfloat16))


def _prep_in_maps(inputs):
    x = np.ascontiguousarray(np.asarray(inputs["x"], dtype=np.float32))
    adj = np.ascontiguousarray(np.asarray(inputs["adj"], dtype=np.float32))
    W_w = np.asarray(inputs["W_w"], dtype=np.float32)
    W_b = np.asarray(inputs["W_b"], dtype=np.float32)
    A = np.asarray(inputs["A"], dtype=np.float32)
    gate_w = np.asarray(inputs["gate_w"], dtype=np.float32)
    gate_b = np.asarray(inputs["gate_b"], dtype=np.float32)

    S = A + A.T
    cb = np.concatenate([
        W_w.T,
        W_w.T @ S,
        np.broadcast_to(gate_w[0, :D].reshape(D, 1), (D, D)),
        np.broadcast_to(gate_w[0, D:].reshape(D, 1), (D, D)),
    ], axis=1)
    cb = _bf16(cb)
    cf = np.ascontiguousarray(
        np.stack([W_b, S @ W_b, np.full(D, gate_b[0])], axis=1),
        dtype=np.float32)

    in_maps = []
    for c in range(NCORES):
        sl = slice(c * BPC, (c + 1) * BPC)
        adj_c = adj[sl]
        adjT_c = _bf16(adj_c.transpose(0, 2, 1))
        xT_c = _bf16(x[sl].transpose(0, 2, 1))
        ndeg = (N - adj_c.sum(axis=1)).astype(np.float32)          # [BPC, N]
        ndegT = np.ascontiguousarray(
            ndeg.reshape(BPC * NB, 128).T)                         # [128, BPC*NB]
        in_maps.append({
            "adjT": adjT_c, "xT": xT_c, "ndegT": ndegT,
            "cb": cb, "cf": cf,
        })
    return in_maps


def _run(inputs, trace=False, **kwargs):
    nc = _get_nc()
    in_maps = _prep_in_maps(inputs)
    res = run_bass_kernel_spmd(nc, in_maps, core_ids=list(range(NCORES)),
                               trace=trace, **kwargs)
    out = np.concatenate(
        [np.asarray(res.results[c]["out"]).astype(np.float32).transpose(0, 2, 1)
         for c in range(NCORES)], axis=0)
    return out, res


def kernel(**inputs) -> np.ndarray:
    out, _ = _run(inputs, trace=False)
    return out


# revision 22
# speedup vs baseline: 1.2717x; 1.0623x over previous
"""Trainium2 Bass kernel for a gated bilinear-attention GNN (GAT-with-gate).

Math (per batch b):
    h   = x @ W_w.T + W_b                      [N, D]
    e   = h (A + A^T) h^T  (symmetrized bilinear score, one quadratic form)
    att = softmax(where(adj>0, e, 0), axis=1) * adj
    rv  = h; 3x: az = relu(att @ rv);  c = sigmoid([h, az] @ gate_w.T + gate_b)
               rv = c * h + (1 - c) * az

Device strategy: data-parallel over the batch dim, 2 batches per core on 8
cores.  v5 design notes:

  * All matmul operands bf16 (f32 PSUM accumulate): 1 row/cycle PE rate.
  * adj ships as bf16 (exact 0/1 mask); x, weights bf16; output bf16
    (host casts to f32).  Measured rel err ~4e-3 vs the 2e-2 gate.
  * hST comes from its own matmul with host-folded WST = W_w.T (A+A^T).
  * Masking happens after the exp: ACT exps e from PSUM into the bf16 attT
    slab; one DVE scalar_tensor_tensor per slab multiplies by adjT in
    place AND accumulates the softmax denominator.  attT is then
    normalized per partition (its partition IS the softmax output index),
    so hops never touch 1/denom and hop 0's stationary is h-natural.
  * The gate matmul uses 128-replicated-column stationaries: sigmoid
    pre-activations appear as 128 identical PSUM rows and ACT applies
    Sigmoid once per batch-hop on [128, N].  In TRANSPOSED space that
    replicated c128 tile IS the broadcast coefficient, so the whole hop
    combine is three full-width bf16 DVE tensor_tensors per batch:
        hmaT = hT - azT;  ttT = hmaT * c128;  rvT = ttT + azT
    and ONE xbar dma transpose turns rvT into the next hop's natural-
    layout stationary.  (ACT does one LUT swap for the whole kernel:
    all att exps precede all sigmoids in ACT program order; Relu lives
    in both tables.)
  * Emission interleaves the two batches op-by-op (engine queues are
    strictly in-order).
  * Data DMAs ride the gpsimd software-DGE queue; the sync queue carries
    only the 8 xbar transposes (~1.3us fixed ucode cost each).

Host side only re-lays-out inputs (shard, transpose, bf16 cast, degree
metadata, folded weights).  _fixup_waits post-processes the scheduled
program to satisfy this walrus build's one-sync-wait-per-instruction limit.
"""

import sys
from contextlib import ExitStack

import numpy as np

sys.path.insert(0, "/opt/trn_rl_repo")

import concourse.bass as bass
import concourse.tile as tile
from concourse import mybir
from concourse.bass_utils import run_bass_kernel_spmd
import concourse.bass_utils as _bu

import ml_dtypes



B, N, D = 16, 1024, 128
NCORES = 8
BPC = B // NCORES        # batches per core
NB = N // 128            # 128-row blocks per matrix dim
F32 = mybir.dt.float32
BF16 = mybir.dt.bfloat16
OP = mybir.AluOpType
AF = mybir.ActivationFunctionType


def build_nc():
    nc = bass.Bass("TRN2", target_bir_lowering=False, debug=False,
                   num_devices=NCORES)

    adjT = nc.dram_tensor("adjT", [BPC, N, N], BF16, kind="ExternalInput").ap()
    xT = nc.dram_tensor("xT", [BPC, D, N], BF16, kind="ExternalInput").ap()
    ndegT = nc.dram_tensor("ndegT", [D, BPC * NB], F32, kind="ExternalInput").ap()
    # packed consts: [WwT | WST | gw1c128 | gw2c128]
    cb = nc.dram_tensor("cb", [D, 4 * D], BF16, kind="ExternalInput").ap()
    # packed f32 consts: [Wb | SWb | gb]
    cf = nc.dram_tensor("cf", [D, 3], F32, kind="ExternalInput").ap()
    out = nc.dram_tensor("out", [BPC, D, N], BF16, kind="ExternalOutput").ap()

    with tile.TileContext(nc) as tc, ExitStack() as ctx:
        consts = ctx.enter_context(tc.tile_pool(name="consts", bufs=1))
        pse = ctx.enter_context(tc.tile_pool(name="pse", bufs=2, space="PSUM"))
        psa = ctx.enter_context(tc.tile_pool(name="psa", bufs=2, space="PSUM"))
        psg = ctx.enter_context(tc.tile_pool(name="psg", bufs=1, space="PSUM"))
        adj_pool = ctx.enter_context(tc.tile_pool(name="adj", bufs=4))
        att_pool = ctx.enter_context(tc.tile_pool(name="att", bufs=2))
        work = ctx.enter_context(tc.tile_pool(name="work", bufs=2))
        hop = ctx.enter_context(tc.tile_pool(name="hop", bufs=3))
        rv_pool = ctx.enter_context(tc.tile_pool(name="rv", bufs=4))

        cb_sb = consts.tile([D, 4 * D], BF16, tag="cb")
        nc.gpsimd.dma_start(cb_sb[:, :], cb[:, :])
        cf_sb = consts.tile([D, 3], F32, tag="cf")
        nc.gpsimd.dma_start(cf_sb[:, :], cf[:, :])
        wwT_sb = cb_sb[:, 0:D]
        wst_sb = cb_sb[:, D:2 * D]
        gw1_sb = cb_sb[:, 2 * D:3 * D]
        gw2_sb = cb_sb[:, 3 * D:4 * D]
        wb_sb = cf_sb[:, 0:1]
        swb_sb = cf_sb[:, 1:2]
        gb_sb = cf_sb[:, 2:3]

        # PE warm-up on the packed const tile: keeps the PE p-state ramped
        # during the DMA-bound startup.
        for _ in range(12):
            wps = psg.tile([128, N], F32, tag="psg")
            nc.tensor.matmul(wps[:, 0:128], wwT_sb[:, :], wwT_sb[:, :],
                             start=True, stop=True)

        states = [{} for _ in range(BPC)]

        def phase_prologue():
            for b in range(BPC):
                st = states[b]
                xT_sb = work.tile([D, N], BF16, tag="xT")
                nc.gpsimd.dma_start(xT_sb[:, :], xT[b, :, :])
                st["xT"] = xT_sb
            ndeg_sb = consts.tile([D, BPC * NB], F32, tag="ndeg")
            nc.gpsimd.dma_start(ndeg_sb[:, :], ndegT[:, :])
            for b in range(BPC):
                st = states[b]
                st["ndeg"] = ndeg_sb[:, b * NB:(b + 1) * NB]
                # hT[o,n] = WwT^T x + Wb ; hST[e,n] = WST^T x + SWb
                hT_sb = work.tile([D, N], BF16, tag="hT")
                hST_sb = work.tile([D, N], BF16, tag="hST")
                for dst, wmat, bias in ((hT_sb, wwT_sb, wb_sb),
                                        (hST_sb, wst_sb, swb_sb)):
                    ph = pse.tile([128, N], F32, tag="pse")
                    for ih in range(2):
                        nc.tensor.matmul(ph[:, ih * 512:(ih + 1) * 512], wmat,
                                         st["xT"][:, ih * 512:(ih + 1) * 512],
                                         start=True, stop=True)
                    nc.scalar.activation(dst[:, :], ph[:, :], AF.Identity,
                                         bias=bias, scale=1.0)
                hnat_sb = work.tile([128, N], BF16, tag="hnat")
                nc.sync.dma_start_transpose(
                    hnat_sb[:, :].rearrange("p (nb f) -> p nb f", nb=NB),
                    hT_sb[:, :])
                st.update(hT=hT_sb, hST=hST_sb, hnat=hnat_sb)

        def att_gen(b):
            # attT[k, j] = adj[j, k] exp(e[k, j]) / denom[k]:
            # exp on ACT straight from PSUM, in-place mask + denominator
            # accumulate on DVE, then per-partition normalize (partition IS
            # the softmax output index).
            st = states[b]
            att_sb = att_pool.tile([128, NB * N], BF16, tag="att")
            acc_sb = work.tile([D, NB], F32, tag="acc")
            inv_sb = work.tile([D, NB], F32, tag="inv")
            st.update(att=att_sb, acc=acc_sb, inv=inv_sb)
            for jb in range(NB):
                adj_sb = adj_pool.tile([128, N], BF16, tag="adj")
                nc.gpsimd.dma_start(adj_sb[:, :],
                                    adjT[b, jb * 128:(jb + 1) * 128, :])
                pe = pse.tile([128, N], F32, tag="pse")
                for ih in range(2):
                    nc.tensor.matmul(pe[:, ih * 512:(ih + 1) * 512],
                                     st["hST"][:, jb * 128:(jb + 1) * 128],
                                     st["hT"][:, ih * 512:(ih + 1) * 512],
                                     start=True, stop=True)
                slab = att_sb[:, jb * N:(jb + 1) * N]
                nc.scalar.activation(slab, pe[:, :], AF.Exp)
                nc.vector.scalar_tensor_tensor(
                    slab, slab, 1.0, adj_sb[:, :], OP.mult, OP.mult,
                    accum_out=acc_sb[:, jb:jb + 1])
                if jb in (3, 7):
                    lo = jb - 3
                    half = slice(lo, jb + 1)
                    nc.vector.tensor_tensor(inv_sb[:, half], acc_sb[:, half],
                                            st["ndeg"][:, half], OP.add)
                    nc.vector.reciprocal(inv_sb[:, half], inv_sb[:, half])
                    for nb in range(lo, jb + 1):
                        nc.vector.tensor_scalar_mul(
                            att_sb[:, nb * N:(nb + 1) * N],
                            att_sb[:, nb * N:(nb + 1) * N],
                            inv_sb[:, nb:nb + 1])
                yield

        def hop_gen(b, k):
            last = (k == 2)
            st = states[b]
            rvs = st.get("rvs") or st["hnat"]
            azT_sb = hop.tile([128, N], BF16, tag="azT")
            for ih in range(2):
                sl = slice(ih * 512, (ih + 1) * 512)
                paz = psa.tile([128, 512], F32, tag="psa")
                for jb in range(NB):
                    nc.tensor.matmul(
                        paz[:, :], rvs[:, jb * 128:(jb + 1) * 128],
                        st["att"][:, jb * N + ih * 512:
                                  jb * N + (ih + 1) * 512],
                        start=(jb == 0), stop=(jb == NB - 1))
                nc.scalar.activation(azT_sb[:, sl], paz[:, :], AF.Relu)
                yield
            # gate + sigmoid + transposed-space combine + xbar, pipelined in
            # N/2 halves so the next hop's first stationary blocks arrive a
            # full half earlier.  c128's replicated rows ARE the broadcast
            # coefficient in transposed space.
            pg = psg.tile([128, N], F32, tag="psg")
            c128_sb = hop.tile([128, N], BF16, tag="c128")
            hma_sb = hop.tile([128, N], BF16, tag="hma")
            tt_sb = hop.tile([128, N], BF16, tag="tt")
            rvT_sb = hop.tile([128, N], BF16, tag="rvT")
            rv_nat = None
            if not last:
                rv_nat = rv_pool.tile([128, N], BF16, tag="rvs")
            nc.vector.tensor_tensor(hma_sb[:, :], st["hT"][:, :],
                                    azT_sb[:, :], OP.subtract)
            for ih in range(2):
                sl = slice(ih * 512, (ih + 1) * 512)
                nc.tensor.matmul(pg[:, sl], gw1_sb, st["hT"][:, sl],
                                 start=True, stop=False)
                nc.tensor.matmul(pg[:, sl], gw2_sb, azT_sb[:, sl],
                                 start=False, stop=True)
                nc.scalar.activation(c128_sb[:, sl], pg[:, sl], AF.Sigmoid,
                                     bias=gb_sb, scale=1.0)
                nc.vector.tensor_tensor(tt_sb[:, sl], hma_sb[:, sl],
                                        c128_sb[:, sl], OP.mult)
                nc.vector.tensor_tensor(rvT_sb[:, sl], tt_sb[:, sl],
                                        azT_sb[:, sl], OP.add)
                if not last:
                    nc.sync.dma_start_transpose(
                        rv_nat[:, sl].rearrange("p (nb f) -> p nb f", nb=4),
                        rvT_sb[:, sl])
                yield
            if last:
                # output stays transposed ([D, N] per batch); host transposes
                nc.gpsimd.dma_start(out[b], rvT_sb[:, :])
            else:
                st["rvs"] = rv_nat
            yield

        def zip_run(*gens):
            gens = list(gens)
            while gens:
                for g in list(gens):
                    try:
                        next(g)
                    except StopIteration:
                        gens.remove(g)

        # software pipeline: att(b1) overlaps hop0(b0); hop(b1,k) overlaps
        # hop(b0,k+1).  Engine queues are in-order, so the overlap comes
        # from round-robin emission of the two phases' chunks.
        phase_prologue()
        zip_run(att_gen(0))
        # hop0(b0)'s matmul/relu chunks overlap att(b1); its sigmoid waits
        # until all att exps are emitted so ACT swaps LUT tables only once.
        ga, gh = att_gen(1), hop_gen(0, 0)
        next(ga); next(gh)
        next(ga); next(gh)
        for _ in range(6):
            next(ga)
        zip_run(ga, gh)
        zip_run(hop_gen(1, 0), hop_gen(0, 1))
        zip_run(hop_gen(1, 1), hop_gen(0, 2))
        zip_run(hop_gen(1, 2))

        # Spare per-engine nops: relocated by _fixup_waits to carry sync
        # waits that walrus cannot fit on compute-instruction structs.
        nop_insts = []
        for eng in (nc.tensor, nc.vector, nc.scalar, nc.gpsimd, nc.sync):
            for _ in range(128):
                nop_insts.append(eng.nop(nofuse=True).ins)

    _fixup_waits(nc, nop_insts)
    return nc


def _fuse_ldweights(nc):
    """Remove the tile framework's pre-split InstLdweights records, merging
    their sync waits/updates into the following (self-loading) Matmult, so
    walrus --enable-ldw-opt=true can overlap stationary loads."""
    f = nc.m.functions[0]
    for blk in f.blocks:
        insts = blk.instructions
        kept = []
        pending = None
        for inst in insts:
            name = inst.__class__.__name__
            if name == "InstLdweights":
                si = inst.sync_info
                assert pending is None
                pending = (list(si.on_wait or []) if si else [],
                           list(si.on_update or []) if si else [])
                continue
            if pending is not None:
                assert name == "InstMatmult", f"ldw followed by {name}"
                w, u = pending
                si = inst.sync_info
                cw = list(si.on_wait or []) if si else []
                cu = list(si.on_update or []) if si else []
                inst.sync_info = mybir.SyncInfo(on_wait=w + cw,
                                                on_update=cu + u)
                pending = None
            kept.append(inst)
        assert pending is None
        if len(kept) != len(insts):
            insts[:] = kept


_FIXUP_SKIP = {"InstNoOp"}


def _fixup_waits(nc, nop_insts):
    """walrus (enable-ldw-opt=false) rejects compute instructions with more
    than one sync wait (single wait slot in the S3 structs).  Hoist
    all-but-one wait of each such instruction onto spare same-engine nop
    instructions inserted immediately before it in program order."""
    nop_set = set(id(x) for x in nop_insts)
    free_nops = {}
    for x in nop_insts:
        free_nops.setdefault(x.engine, []).append(x)
    f = nc.m.functions[0]
    for blk in f.blocks:
        insts = blk.instructions
        for i in range(len(insts) - 1, -1, -1):
            if id(insts[i]) in nop_set:
                insts.pop(i)
        i = 0
        while i < len(insts):
            inst = insts[i]
            if inst.__class__.__name__ not in _FIXUP_SKIP:
                si = inst.sync_info
                if si is not None and si.on_wait and len(si.on_wait) > 1:
                    waits = list(si.on_wait)
                    extra, keep = waits[:-1], waits[-1:]
                    inst.sync_info = mybir.SyncInfo(
                        on_wait=keep, on_update=list(si.on_update or []))
                    pool = free_nops.get(inst.engine)
                    for kk, w in enumerate(extra):
                        if not pool:
                            raise RuntimeError(
                                f"out of spare nops for {inst.engine}")
                        nop = pool.pop()
                        nop.sync_info = mybir.SyncInfo(on_wait=[w], on_update=[])
                        insts.insert(i + kk, nop)
                    i += len(extra)
            i += 1


_NC_CACHE = None


def _get_nc():
    global _NC_CACHE
    if _NC_CACHE is None:
        _NC_CACHE = build_nc()
    return _NC_CACHE


def _bf16(a):
    return np.ascontiguousarray(
        np.asarray(a, dtype=np.float32).astype(ml_dtypes.bfloat16))


def _prep_in_maps(inputs):
    x = np.ascontiguousarray(np.asarray(inputs["x"], dtype=np.float32))
    adj = np.ascontiguousarray(np.asarray(inputs["adj"], dtype=np.float32))
    W_w = np.asarray(inputs["W_w"], dtype=np.float32)
    W_b = np.asarray(inputs["W_b"], dtype=np.float32)
    A = np.asarray(inputs["A"], dtype=np.float32)
    gate_w = np.asarray(inputs["gate_w"], dtype=np.float32)
    gate_b = np.asarray(inputs["gate_b"], dtype=np.float32)

    S = A + A.T
    cb = np.concatenate([
        W_w.T,
        W_w.T @ S,
        np.broadcast_to(gate_w[0, :D].reshape(D, 1), (D, D)),
        np.broadcast_to(gate_w[0, D:].reshape(D, 1), (D, D)),
    ], axis=1)
    cb = _bf16(cb)
    cf = np.ascontiguousarray(
        np.stack([W_b, S @ W_b, np.full(D, gate_b[0])], axis=1),
        dtype=np.float32)

    in_maps = []
    for c in range(NCORES):
        sl = slice(c * BPC, (c + 1) * BPC)
        adj_c = adj[sl]
        adjT_c = _bf16(adj_c.transpose(0, 2, 1))
        xT_c = _bf16(x[sl].transpose(0, 2, 1))
        ndeg = (N - adj_c.sum(axis=1)).astype(np.float32)          # [BPC, N]
        ndegT = np.ascontiguousarray(
            ndeg.reshape(BPC * NB, 128).T)                         # [128, BPC*NB]
        in_maps.append({
            "adjT": adjT_c, "xT": xT_c, "ndegT": ndegT,
            "cb": cb, "cf": cf,
        })
    return in_maps


def _run(inputs, trace=False, **kwargs):
    nc = _get_nc()
    in_maps = _prep_in_maps(inputs)
    res = run_bass_kernel_spmd(nc, in_maps, core_ids=list(range(NCORES)),
                               trace=trace, **kwargs)
    out = np.concatenate(
        [np.asarray(res.results[c]["out"]).astype(np.float32).transpose(0, 2, 1)
         for c in range(NCORES)], axis=0)
    return out, res


def kernel(**inputs) -> np.ndarray:
    out, _ = _run(inputs, trace=False)
    return out
